# revision 97
# baseline (speedup 1.0000x reference)
"""MoE LoRA delta kernel for Trainium2 (8 NeuronCores, data-parallel over tokens).

Computation (per token t):
    logits = x @ router_w.T                      [T, 4]
    gates  = top2-softmax(logits)                [T, 4]  (exactly 2 nonzero)
    mid    = x @ A_all.T                         [T, 64]   A_all[(e,r), d]
    delta  = (mid * expand(gates) * 4.0) @ B_all [T, D]    B_all[(e,r), d]

Kernel strategy per core (T_c = 1024 tokens, 4 groups of 256):
  - W = concat([A_all, router_w]) -> [68, D]; host packs W.T into the SBUF
    partition layout [128, 30*68] so the weight DMA moves 8KB descriptors.
  - x.T tiles produced on-chip with PE transpose-mode matmuls (fp32, exact),
    evacuated to SBUF as float32r so mm1 runs at 1 cycle/row.
  - mm1 computes [68, 256] = W @ x.T in f32r with fp32 PSUM accumulation;
    rows 64:68 are the router logits.  f32r keeps the logits accurate enough
    that the top-2 expert selection matches the fp32 reference (bf16/fp16 x
    flips 2-10 tokens on the actual dataset, each costing ~0.4 rel err, so
    x must stay 4 bytes; this pins the DMA roofline at ~92us/core).
  - Gating runs in fp32 on the accumulated logits: g_e = 1{t_e >= m2} *
    sigmoid(2*t_e - m2), t = l - max(l).
  - mm2 runs in fp16 (same 1 cycle/row on PE, halves the B_all weight DMA,
    11-bit mantissa); products accumulate in fp32 PSUM.  The output is
    stored as fp16 (halves the store DMA) and the host gather upcasts to
    fp32; measured rel err 6.8e-4 against the 2e-2 budget.
  - Schedule: the next group's transpose tiles are interleaved between this
    group's mm1/gating and mm2 so the PE never waits on the DVE/ACT gating
    chain; PSUM->SBUF evacuations alternate ACT-first/DVE; outputs are
    stored as two half-tile DMAs per 128-token tile.
  - DMA: x loads + weights on the SP HWDGE queue, output stores on the Pool
    SWDGE queue, so the (serialized, 360 GB/s) DMA engines see back-to-back
    transfers from two independent queues: zero idle between the first and
    last transfer in the cost-model timeline.
"""

import os
import sys

for _p in ("/opt/trn_rl_repo", "/root/.axon_site/_ro/trn_rl_repo"):
    if os.path.isdir(_p) and _p not in sys.path:
        sys.path.insert(0, _p)

import numpy as np
from contextlib import ExitStack

import concourse.bass as bass
import concourse.bacc as bacc
import concourse.mybir as mybir
import concourse.tile as tile

N_CORES = 8
B_, S, D = 4, 2048, 3840
T_FULL = B_ * S                 # 8192
T_C = T_FULL // N_CORES         # 1024 tokens per core
E, R = 4, 16
ER = E * R                      # 64
M_W = ER + E                    # 68 = A rows + router rows
LORA_SCALE = 16.0 / np.sqrt(16.0)   # 4.0

GROUP = 256                     # tokens per mm1 group
TPG = GROUP // 128              # token tiles per group (2)
N_GROUPS = T_C // GROUP         # 4
D_CHUNKS = D // 128             # 30
MM2_CHUNKS = [(i * 512, min(512, D - i * 512)) for i in range((D + 511) // 512)]

F32 = mybir.dt.float32
F32R = mybir.dt.float32r

BF16 = mybir.dt.bfloat16
F16 = mybir.dt.float16

# Dtype choices:
#  - mm1 f32r: 1 cyc/row on PE and keeps the router logits at ~19-bit
#    precision so the top-2 expert selection matches the fp32 reference.
#  - mm2 fp16: same PE speed, halves the B_all weight DMA, 11-bit mantissa.
#  - output fp16 on device (halves the store DMA -- the single biggest
#    traffic item after x); the host gather upcasts to fp32.  Quantization
#    adds ~5e-4 rel err against the 2e-2 budget.
_DT_MAP = {"f32": F32, "f32r": F32R, "bf16": BF16, "f16": F16}
MM1_DT = _DT_MAP[os.environ.get("MOE_MM1", "f32r")]
MM2_DT = _DT_MAP[os.environ.get("MOE_MM2", "f16")]
OUT_DT = _DT_MAP[os.environ.get("MOE_OUT", "f16")]
# x streams through the PE transposes as f32r: 1.5 cycles/row vs 2.0 for
# fp32, and numerically identical here (xt is f32r-rounded downstream
# anyway).  The identity must match (walrus rejects mixed 32/16-bit matmul
# inputs, and fp32+anything is broken in HW).
X_DT = _DT_MAP[os.environ.get("MOE_X", "f32r")]
ID_DT = _DT_MAP[os.environ.get("MOE_ID", "f32r")]


def build_kernel(tc: tile.TileContext, out_d, x_d, wt_d, b_d, sel_d, id_d,
                 id32_d):
    nc = tc.nc
    with ExitStack() as ctx:
        const_pool = ctx.enter_context(tc.tile_pool(name="const", bufs=1))
        xin_pool = ctx.enter_context(tc.tile_pool(name="xin", bufs=4))
        xt_pool = ctx.enter_context(tc.tile_pool(name="xt", bufs=2))
        mid_pool = ctx.enter_context(tc.tile_pool(name="mid", bufs=2))
        g_pool = ctx.enter_context(tc.tile_pool(name="gate", bufs=2))
        dout_pool = ctx.enter_context(tc.tile_pool(name="dout", bufs=8))
        ps_tp = ctx.enter_context(
            tc.tile_pool(name="ps_tp", bufs=2, space=bass.MemorySpace.PSUM))
        ps_mm1 = ctx.enter_context(
            tc.tile_pool(name="ps_mm1", bufs=2, space=bass.MemorySpace.PSUM))
        ps_g = ctx.enter_context(
            tc.tile_pool(name="ps_g", bufs=1, space=bass.MemorySpace.PSUM))
        ps_mm2 = ctx.enter_context(
            tc.tile_pool(name="ps_mm2", bufs=3, space=bass.MemorySpace.PSUM))

        PROC = list(range(N_GROUPS))
        PROC_TILES = [g * TPG + tl for g in PROC for tl in range(TPG)]

        # ---- prologue DMAs (SP queue): first x tile first so PE starts early
        x_sb = {}

        def load_x_seq(seq):
            t = PROC_TILES[seq]
            x_sb[t] = xin_pool.tile([128, D], X_DT, tag="xin", name=f"x_t{t}")
            nc.sync.dma_start(x_sb[t][:], x_d[t * 128:(t + 1) * 128, :])

        # Issue ALL loads up front: the xin pool's WAR deps throttle them to
        # the transpose consumption pace, and a parked load always beats a
        # later-arriving store to the DMA engines, so loads finish early and
        # the back half of the bus schedule is pure stores.
        load_x_seq(0)
        id_sb = const_pool.tile([128, 128], ID_DT, tag="ident")
        nc.sync.dma_start(id_sb[:], id_d[:])
        # fp32 identity for the small gating transposes (their data is fp32)
        id32_sb = const_pool.tile([128, 128], F32, tag="ident32")
        nc.sync.dma_start(id32_sb[:], id32_d[:])
        wt_sb = const_pool.tile([128, D_CHUNKS, M_W], MM1_DT, tag="wt")
        nc.sync.dma_start(wt_sb[:], wt_d.rearrange("p (c m) -> p c m", m=M_W))
        load_x_seq(1)
        b_sb = const_pool.tile([ER, D], MM2_DT, tag="ball")
        nc.sync.dma_start(b_sb[:], b_d[:])
        sel_sb = const_pool.tile([E, ER], MM2_DT, tag="sel")
        nc.sync.dma_start(sel_sb[:], sel_d[:])
        for _seq in range(2, N_GROUPS * TPG):
            load_x_seq(_seq)

        cp_engines = [nc.scalar, nc.vector]
        xt_i = 0
        do_i = 0

        TPC = 4   # chunks per transpose-evacuation copy

        def new_xt(g):
            return xt_pool.tile(
                [128, D_CHUNKS, GROUP], MM1_DT, tag="xt", name=f"xt_g{g}")

        def emit_transpose_tile(xt_sb, gi, tl):
            """One 128-token x tile (processing position gi, tile tl) ->
            xt_sb (f32r), PE transposes + DVE/ACT evacuation."""
            nonlocal xt_i
            seq = gi * TPG + tl
            t = PROC_TILES[seq]
            for c0 in range(0, D_CHUNKS, TPC):
                ncc = min(TPC, D_CHUNKS - c0)
                tp_ps = ps_tp.tile([128, TPC, 128], X_DT, tag="tp")
                for cc in range(ncc):
                    c = c0 + cc
                    nc.tensor.transpose(
                        tp_ps[:, cc, :],
                        x_sb[t][:, c * 128:(c + 1) * 128],
                        id_sb[:],
                    )
                eng = cp_engines[xt_i % 2]; xt_i += 1
                dst = xt_sb[:, c0:c0 + ncc, tl * 128:(tl + 1) * 128]
                if eng is nc.vector:
                    eng.tensor_copy(dst, tp_ps[:, 0:ncc, :])
                else:
                    eng.copy(dst, tp_ps[:, 0:ncc, :])

        DSPLIT = 2048   # store each tile as two half-tile DMAs

        def emit_mm2_tile(midTs, tok_g_p, tl, store_q=None):
            # Early groups store on the SP queue: strict FIFO behind every
            # load, so stores can never steal bus slots mid-load-stream
            # (arrival-order arbitration otherwise stretches the last load
            # by ~25us).  Only the final group's stores use the Pool queue.
            nonlocal do_i
            store_q = store_q or nc.sync
            tok0 = tok_g_p + tl * 128
            dout_sb = dout_pool.tile([128, D], OUT_DT, tag="dout")
            for (d0, w) in MM2_CHUNKS:
                mm2_ps = ps_mm2.tile([128, 512], F32, tag="mm2")
                nc.tensor.matmul(
                    mm2_ps[:, 0:w],
                    midTs[:, tl * 128:(tl + 1) * 128],
                    b_sb[:, d0:d0 + w],
                )
                eng = cp_engines[do_i % 2]; do_i += 1
                if eng is nc.vector:
                    eng.tensor_copy(dout_sb[:, d0:d0 + w], mm2_ps[:, 0:w])
                else:
                    eng.copy(dout_sb[:, d0:d0 + w], mm2_ps[:, 0:w])
                if d0 + w == DSPLIT:
                    store_q.dma_start(
                        out_d[tok0:tok0 + 128, 0:DSPLIT], dout_sb[:, 0:DSPLIT])
            store_q.dma_start(
                out_d[tok0:tok0 + 128, DSPLIT:D], dout_sb[:, DSPLIT:D])

        xt_cur = new_xt(PROC[0])
        for tl in range(TPG):
            emit_transpose_tile(xt_cur, 0, tl)

        for gi, g in enumerate(PROC):
            tok_g = g * GROUP

            # ---- mm1: [68, GROUP] = W @ x.T (f32r, fp32 accumulation) ----
            mid_ps = ps_mm1.tile([M_W, GROUP], F32, tag="mm1")
            for c in range(D_CHUNKS):
                nc.tensor.matmul(
                    mid_ps[:],
                    wt_sb[:, c, :],
                    xt_cur[:, c, :],
                    start=(c == 0),
                    stop=(c == D_CHUNKS - 1),
                )

            # ---- gating (fp32, tokens on partitions) ----
            lg_sb = g_pool.tile([M_W, GROUP], F32, tag="lg")
            nc.vector.tensor_copy(lg_sb[ER:M_W, :], mid_ps[ER:M_W, :])

            logT_ps = ps_g.tile([128, TPG, E], F32, tag="gps")
            for tl in range(TPG):
                nc.tensor.matmul(
                    logT_ps[:, tl, :],
                    lg_sb[ER:M_W, tl * 128:(tl + 1) * 128],
                    id32_sb[ER:M_W, ER:M_W],
                    is_transpose=True,
                )

            gates_sb = g_pool.tile([128, TPG, E], F32, tag="gates")
            for tl in range(TPG):
                L = g_pool.tile([128, E], F32, tag="L")
                nc.vector.tensor_copy(L[:], logT_ps[:, tl, :])
                m1 = g_pool.tile([128, 1], F32, tag="m1")
                nc.vector.tensor_reduce(
                    m1[:], L[:], axis=mybir.AxisListType.X, op=mybir.AluOpType.max)
                tt = g_pool.tile([128, E], F32, tag="tt")
                nc.vector.tensor_scalar(
                    tt[:], L[:], m1[:], None, op0=mybir.AluOpType.subtract)
                z = g_pool.tile([128, E], F32, tag="z")
                nc.vector.tensor_scalar(
                    z[:], tt[:], 0.0, None, op0=mybir.AluOpType.is_equal)
                msk = g_pool.tile([128, E], F32, tag="msk")
                nc.vector.scalar_tensor_tensor(
                    msk[:], z[:], -1e30, tt[:],
                    op0=mybir.AluOpType.mult, op1=mybir.AluOpType.add)
                m2 = g_pool.tile([128, 1], F32, tag="m2")
                nc.vector.tensor_reduce(
                    m2[:], msk[:], axis=mybir.AxisListType.X, op=mybir.AluOpType.max)
                s2 = g_pool.tile([128, E], F32, tag="s2")
                nc.vector.tensor_scalar(
                    s2[:], tt[:], 2.0, m2[:],
                    op0=mybir.AluOpType.mult, op1=mybir.AluOpType.subtract)
                sg = g_pool.tile([128, E], F32, tag="sg")
                nc.scalar.activation(
                    sg[:], s2[:], mybir.ActivationFunctionType.Sigmoid)
                ge = g_pool.tile([128, E], F32, tag="ge")
                nc.vector.tensor_scalar(
                    ge[:], tt[:], m2[:], None, op0=mybir.AluOpType.is_ge)
                nc.vector.tensor_tensor(
                    gates_sb[:, tl, :], ge[:], sg[:], op=mybir.AluOpType.mult)

            # ---- next group's first transpose tile fills PE while gating runs
            if gi + 1 < N_GROUPS:
                xt_next = new_xt(PROC[gi + 1])
                emit_transpose_tile(xt_next, gi + 1, 0)

            # ---- gates -> (e,r)-expanded scale -> midTs ----
            gT_ps = ps_g.tile([E, GROUP], F32, tag="gps")
            for tl in range(TPG):
                nc.tensor.matmul(
                    gT_ps[:, tl * 128:(tl + 1) * 128],
                    gates_sb[:, tl, :],
                    id32_sb[:],
                    is_transpose=True,
                )
            gT_sb = g_pool.tile([E, GROUP], MM2_DT, tag="gT")
            nc.vector.tensor_copy(gT_sb[:], gT_ps[:])

            gexp_ps = ps_g.tile([ER, GROUP], F32, tag="gps")
            nc.tensor.matmul(gexp_ps[:], sel_sb[:], gT_sb[:])
            gexp_sb = g_pool.tile([ER, GROUP], F32, tag="gexp")
            nc.scalar.copy(gexp_sb[:], gexp_ps[:])

            midTs = mid_pool.tile([ER, GROUP], MM2_DT, tag="midTs")
            nc.vector.tensor_tensor(
                midTs[:], mid_ps[0:ER, :], gexp_sb[:], op=mybir.AluOpType.mult)

            # ---- mm2 + stores for THIS group, interleaved with the next
            # group's remaining transpose tile ----
            sq = nc.sync if gi + 1 < N_GROUPS else nc.gpsimd
            emit_mm2_tile(midTs, tok_g, 0, store_q=sq)
            if gi + 1 < N_GROUPS:
                emit_transpose_tile(xt_next, gi + 1, 1)
            emit_mm2_tile(midTs, tok_g, 1, store_q=sq)

            if gi + 1 < N_GROUPS:
                xt_cur = xt_next


_CACHED = {}


def _build_module():
    key = (MM1_DT, MM2_DT, OUT_DT)
    if key in _CACHED:
        return _CACHED[key]
    nc = bacc.Bacc("TRN2", target_bir_lowering=False, debug=False)
    x_d = nc.dram_tensor("x_in", [T_C, D], X_DT, kind="ExternalInput").ap()
    wt_d = nc.dram_tensor(
        "wt_in", [128, D_CHUNKS * M_W], MM1_DT, kind="ExternalInput").ap()
    b_d = nc.dram_tensor("ball_in", [ER, D], MM2_DT, kind="ExternalInput").ap()
    sel_d = nc.dram_tensor("sel_in", [E, ER], MM2_DT, kind="ExternalInput").ap()
    id_d = nc.dram_tensor("id_in", [128, 128], ID_DT, kind="ExternalInput").ap()
    id32_d = nc.dram_tensor(
        "id32_in", [128, 128], F32, kind="ExternalInput").ap()
    out_d = nc.dram_tensor("out", [T_C, D], OUT_DT, kind="ExternalOutput").ap()
    with tile.TileContext(nc) as tc:
        build_kernel(tc, out_d, x_d, wt_d, b_d, sel_d, id_d, id32_d)
    nc.compile()
    _CACHED[key] = nc
    return nc


def _host_weights(router_w, A, B):
    W = np.concatenate([A.reshape(ER, D), router_w], axis=0).astype(np.float32)
    # pack W.T [D, 68] into SBUF partition layout [128, 30*68]:
    # partition p, chunk c, row m  <-  W.T[c*128+p, m]
    WT = np.ascontiguousarray(
        W.T.reshape(D_CHUNKS, 128, M_W).transpose(1, 0, 2).reshape(
            128, D_CHUNKS * M_W))
    B_all = np.ascontiguousarray(
        B.transpose(0, 2, 1).reshape(ER, D)).astype(np.float32)      # [(e,r), d]
    sel = np.zeros((E, ER), np.float32)
    for e in range(E):
        sel[e, e * R:(e + 1) * R] = LORA_SCALE
    import ml_dtypes
    _np_map = {F32: np.float32, F32R: np.float32,
               BF16: ml_dtypes.bfloat16, F16: np.float16}
    ident32 = np.eye(128, dtype=np.float32)
    ident = ident32.astype(_np_map[ID_DT])
    B_all = B_all.astype(_np_map[MM2_DT])
    sel = sel.astype(_np_map[MM2_DT])
    return WT, B_all, sel, ident, ident32


def make_in_maps(x, router_w, A, B):
    flat = np.ascontiguousarray(np.asarray(x, np.float32).reshape(T_FULL, D))
    WT, B_all, sel, ident, ident32 = _host_weights(
        np.asarray(router_w, np.float32),
        np.asarray(A, np.float32),
        np.asarray(B, np.float32))
    in_maps = []
    for i in range(N_CORES):
        in_maps.append({
            "x_in": flat[i * T_C:(i + 1) * T_C],
            "wt_in": WT,
            "ball_in": B_all,
            "sel_in": sel,
            "id_in": ident,
            "id32_in": ident32,
        })
    return in_maps


def kernel(x, router_w, A, B, _results_hook=None):
    from concourse.bass_utils import run_bass_kernel_spmd

    nc = _build_module()
    in_maps = make_in_maps(x, router_w, A, B)
    res = run_bass_kernel_spmd(nc, in_maps, core_ids=list(range(N_CORES)))
    if _results_hook is not None:
        _results_hook(res)
    out = np.concatenate([res.results[i]["out"] for i in range(N_CORES)], axis=0)
    return out.astype(np.float32, copy=False).reshape(B_, S, D)


if __name__ == "__main__":
    rng = np.random.default_rng(0)
    x = rng.standard_normal((B_, S, D), dtype=np.float32)
    rw = (rng.standard_normal((E, D)) * 0.02).astype(np.float32)
    A = (rng.standard_normal((E, R, D)) * 0.02).astype(np.float32)
    Bm = (rng.standard_normal((E, D, R)) * 0.02).astype(np.float32)
    out = kernel(x, rw, A, Bm)
    print("out", out.shape, out.dtype, float(np.abs(out).max()))


# revision 99
# speedup vs baseline: 1.0065x; 1.0065x over previous
"""MoE LoRA delta kernel for Trainium2 (8 NeuronCores, data-parallel over tokens).

Computation (per token t):
    logits = x @ router_w.T                      [T, 4]
    gates  = top2-softmax(logits)                [T, 4]  (exactly 2 nonzero)
    mid    = x @ A_all.T                         [T, 64]   A_all[(e,r), d]
    delta  = (mid * expand(gates) * 4.0) @ B_all [T, D]    B_all[(e,r), d]

Kernel strategy per core (T_c = 1024 tokens, 4 groups of 256):
  - W = concat([A_all, router_w]) -> [68, D]; host packs W.T into the SBUF
    partition layout [128, 30*68] so the weight DMA moves 8KB descriptors.
  - x.T tiles produced on-chip with PE transpose-mode matmuls (fp32, exact),
    evacuated to SBUF as float32r so mm1 runs at 1 cycle/row.
  - mm1 computes [68, 256] = W @ x.T in f32r with fp32 PSUM accumulation;
    rows 64:68 are the router logits.  f32r keeps the logits accurate enough
    that the top-2 expert selection matches the fp32 reference (bf16/fp16 x
    flips 2-10 tokens on the actual dataset, each costing ~0.4 rel err, so
    x must stay 4 bytes; this pins the DMA roofline at ~92us/core).
  - Gating runs in fp32 on the accumulated logits: g_e = 1{t_e >= m2} *
    sigmoid(2*t_e - m2), t = l - max(l).
  - mm2 runs in fp16 (same 1 cycle/row on PE, halves the B_all weight DMA,
    11-bit mantissa); products accumulate in fp32 PSUM.  The output is
    stored as fp16 (halves the store DMA) and the host gather upcasts to
    fp32; measured rel err 6.8e-4 against the 2e-2 budget.
  - Schedule: the next group's transpose tiles are interleaved between this
    group's mm1/gating and mm2 so the PE never waits on the DVE/ACT gating
    chain; PSUM->SBUF evacuations alternate ACT-first/DVE; outputs are
    stored as two half-tile DMAs per 128-token tile.
  - DMA: x loads + weights on the SP HWDGE queue, output stores on the Pool
    SWDGE queue, so the (serialized, 360 GB/s) DMA engines see back-to-back
    transfers from two independent queues: zero idle between the first and
    last transfer in the cost-model timeline.
"""

import os
import sys

for _p in ("/opt/trn_rl_repo", "/root/.axon_site/_ro/trn_rl_repo"):
    if os.path.isdir(_p) and _p not in sys.path:
        sys.path.insert(0, _p)

import numpy as np
from contextlib import ExitStack

import concourse.bass as bass
import concourse.bacc as bacc
import concourse.mybir as mybir
import concourse.tile as tile

N_CORES = 8
B_, S, D = 4, 2048, 3840
T_FULL = B_ * S                 # 8192
T_C = T_FULL // N_CORES         # 1024 tokens per core
E, R = 4, 16
ER = E * R                      # 64
M_W = ER + E                    # 68 = A rows + router rows
LORA_SCALE = 16.0 / np.sqrt(16.0)   # 4.0

GROUP = 256                     # tokens per mm1 group
TPG = GROUP // 128              # token tiles per group (2)
N_GROUPS = T_C // GROUP         # 4
D_CHUNKS = D // 128             # 30
MM2_CHUNKS = [(i * 512, min(512, D - i * 512)) for i in range((D + 511) // 512)]

F32 = mybir.dt.float32
F32R = mybir.dt.float32r

BF16 = mybir.dt.bfloat16
F16 = mybir.dt.float16

# Dtype choices:
#  - mm1 f32r: 1 cyc/row on PE and keeps the router logits at ~19-bit
#    precision so the top-2 expert selection matches the fp32 reference.
#  - mm2 fp16: same PE speed, halves the B_all weight DMA, 11-bit mantissa.
#  - output fp16 on device (halves the store DMA -- the single biggest
#    traffic item after x); the host gather upcasts to fp32.  Quantization
#    adds ~5e-4 rel err against the 2e-2 budget.
_DT_MAP = {"f32": F32, "f32r": F32R, "bf16": BF16, "f16": F16}
MM1_DT = _DT_MAP[os.environ.get("MOE_MM1", "f32r")]
MM2_DT = _DT_MAP[os.environ.get("MOE_MM2", "f16")]
OUT_DT = _DT_MAP[os.environ.get("MOE_OUT", "f16")]
# x streams through the PE transposes as f32r: 1.5 cycles/row vs 2.0 for
# fp32, and numerically identical here (xt is f32r-rounded downstream
# anyway).  The identity must match (walrus rejects mixed 32/16-bit matmul
# inputs, and fp32+anything is broken in HW).
X_DT = _DT_MAP[os.environ.get("MOE_X", "f32r")]
ID_DT = _DT_MAP[os.environ.get("MOE_ID", "f32r")]


def build_kernel(tc: tile.TileContext, out_d, x_d, wt_d, b_d, sel_d, id_d,
                 id32_d):
    nc = tc.nc
    with ExitStack() as ctx:
        const_pool = ctx.enter_context(tc.tile_pool(name="const", bufs=1))
        xin_pool = ctx.enter_context(tc.tile_pool(name="xin", bufs=4))
        xt_pool = ctx.enter_context(tc.tile_pool(name="xt", bufs=2))
        mid_pool = ctx.enter_context(tc.tile_pool(name="mid", bufs=2))
        g_pool = ctx.enter_context(tc.tile_pool(name="gate", bufs=2))
        dout_pool = ctx.enter_context(tc.tile_pool(name="dout", bufs=8))
        ps_tp = ctx.enter_context(
            tc.tile_pool(name="ps_tp", bufs=2, space=bass.MemorySpace.PSUM))
        ps_mm1 = ctx.enter_context(
            tc.tile_pool(name="ps_mm1", bufs=2, space=bass.MemorySpace.PSUM))
        ps_g = ctx.enter_context(
            tc.tile_pool(name="ps_g", bufs=1, space=bass.MemorySpace.PSUM))
        ps_mm2 = ctx.enter_context(
            tc.tile_pool(name="ps_mm2", bufs=3, space=bass.MemorySpace.PSUM))

        PROC = list(range(N_GROUPS))
        PROC_TILES = [g * TPG + tl for g in PROC for tl in range(TPG)]

        # ---- prologue DMAs (SP queue): first x tile first so PE starts early
        x_sb = {}

        def load_x_seq(seq):
            t = PROC_TILES[seq]
            x_sb[t] = xin_pool.tile([128, D], X_DT, tag="xin", name=f"x_t{t}")
            nc.sync.dma_start(x_sb[t][:], x_d[t * 128:(t + 1) * 128, :])

        # Issue ALL loads up front: the xin pool's WAR deps throttle them to
        # the transpose consumption pace, and a parked load always beats a
        # later-arriving store to the DMA engines, so loads finish early and
        # the back half of the bus schedule is pure stores.
        load_x_seq(0)
        id_sb = const_pool.tile([128, 128], ID_DT, tag="ident")
        nc.sync.dma_start(id_sb[:], id_d[:])
        # fp32 identity for the small gating transposes (their data is fp32)
        id32_sb = const_pool.tile([128, 128], F32, tag="ident32")
        nc.sync.dma_start(id32_sb[:], id32_d[:])
        wt_sb = const_pool.tile([128, D_CHUNKS, M_W], MM1_DT, tag="wt")
        nc.sync.dma_start(wt_sb[:], wt_d.rearrange("p (c m) -> p c m", m=M_W))
        load_x_seq(1)
        b_sb = const_pool.tile([ER, D], MM2_DT, tag="ball")
        nc.sync.dma_start(b_sb[:], b_d[:])
        sel_sb = const_pool.tile([E, ER], MM2_DT, tag="sel")
        nc.sync.dma_start(sel_sb[:], sel_d[:])
        for _seq in range(2, N_GROUPS * TPG):
            load_x_seq(_seq)

        cp_engines = [nc.scalar, nc.vector]
        xt_i = 0
        do_i = 0

        TPC = 4   # chunks per transpose-evacuation copy

        def new_xt(g):
            return xt_pool.tile(
                [128, D_CHUNKS, GROUP], MM1_DT, tag="xt", name=f"xt_g{g}")

        def emit_transpose_tile(xt_sb, gi, tl):
            """One 128-token x tile (processing position gi, tile tl) ->
            xt_sb (f32r), PE transposes + DVE/ACT evacuation."""
            nonlocal xt_i
            seq = gi * TPG + tl
            t = PROC_TILES[seq]
            for c0 in range(0, D_CHUNKS, TPC):
                ncc = min(TPC, D_CHUNKS - c0)
                tp_ps = ps_tp.tile([128, TPC, 128], X_DT, tag="tp")
                for cc in range(ncc):
                    c = c0 + cc
                    nc.tensor.transpose(
                        tp_ps[:, cc, :],
                        x_sb[t][:, c * 128:(c + 1) * 128],
                        id_sb[:],
                    )
                eng = cp_engines[xt_i % 2]; xt_i += 1
                dst = xt_sb[:, c0:c0 + ncc, tl * 128:(tl + 1) * 128]
                if eng is nc.vector:
                    eng.tensor_copy(dst, tp_ps[:, 0:ncc, :])
                else:
                    eng.copy(dst, tp_ps[:, 0:ncc, :])

        DSPLIT = 2048   # store each tile as two half-tile DMAs

        def emit_mm2_tile(midTs, tok_g_p, tl, store_q=None):
            # Early groups store on the SP queue: strict FIFO behind every
            # load, so stores can never steal bus slots mid-load-stream
            # (arrival-order arbitration otherwise stretches the last load
            # by ~25us).  Only the final group's stores use the Pool queue.
            nonlocal do_i
            store_q = store_q or nc.sync
            tok0 = tok_g_p + tl * 128
            dout_sb = dout_pool.tile([128, D], OUT_DT, tag="dout")
            for (d0, w) in MM2_CHUNKS:
                mm2_ps = ps_mm2.tile([128, 512], F32, tag="mm2")
                nc.tensor.matmul(
                    mm2_ps[:, 0:w],
                    midTs[:, tl * 128:(tl + 1) * 128],
                    b_sb[:, d0:d0 + w],
                )
                eng = cp_engines[do_i % 2]; do_i += 1
                if eng is nc.vector:
                    eng.tensor_copy(dout_sb[:, d0:d0 + w], mm2_ps[:, 0:w])
                else:
                    eng.copy(dout_sb[:, d0:d0 + w], mm2_ps[:, 0:w])
                if d0 + w == DSPLIT:
                    store_q.dma_start(
                        out_d[tok0:tok0 + 128, 0:DSPLIT], dout_sb[:, 0:DSPLIT])
            store_q.dma_start(
                out_d[tok0:tok0 + 128, DSPLIT:D], dout_sb[:, DSPLIT:D])

        xt_cur = new_xt(PROC[0])
        for tl in range(TPG):
            emit_transpose_tile(xt_cur, 0, tl)

        for gi, g in enumerate(PROC):
            tok_g = g * GROUP

            # ---- mm1: [68, GROUP] = W @ x.T (f32r, fp32 accumulation) ----
            mid_ps = ps_mm1.tile([M_W, GROUP], F32, tag="mm1")
            for c in range(D_CHUNKS):
                nc.tensor.matmul(
                    mid_ps[:],
                    wt_sb[:, c, :],
                    xt_cur[:, c, :],
                    start=(c == 0),
                    stop=(c == D_CHUNKS - 1),
                )

            # ---- gating (fp32, tokens on partitions) ----
            lg_sb = g_pool.tile([M_W, GROUP], F32, tag="lg")
            nc.vector.tensor_copy(lg_sb[ER:M_W, :], mid_ps[ER:M_W, :])

            logT_ps = ps_g.tile([128, TPG, E], F32, tag="gps")
            for tl in range(TPG):
                nc.tensor.matmul(
                    logT_ps[:, tl, :],
                    lg_sb[ER:M_W, tl * 128:(tl + 1) * 128],
                    id32_sb[ER:M_W, ER:M_W],
                    is_transpose=True,
                )

            gates_sb = g_pool.tile([128, TPG, E], F32, tag="gates")
            for tl in range(TPG):
                L = g_pool.tile([128, E], F32, tag="L")
                nc.vector.tensor_copy(L[:], logT_ps[:, tl, :])
                m1 = g_pool.tile([128, 1], F32, tag="m1")
                nc.vector.tensor_reduce(
                    m1[:], L[:], axis=mybir.AxisListType.X, op=mybir.AluOpType.max)
                tt = g_pool.tile([128, E], F32, tag="tt")
                nc.vector.tensor_scalar(
                    tt[:], L[:], m1[:], None, op0=mybir.AluOpType.subtract)
                z = g_pool.tile([128, E], F32, tag="z")
                nc.vector.tensor_scalar(
                    z[:], tt[:], 0.0, None, op0=mybir.AluOpType.is_equal)
                msk = g_pool.tile([128, E], F32, tag="msk")
                nc.vector.scalar_tensor_tensor(
                    msk[:], z[:], -1e30, tt[:],
                    op0=mybir.AluOpType.mult, op1=mybir.AluOpType.add)
                m2 = g_pool.tile([128, 1], F32, tag="m2")
                nc.vector.tensor_reduce(
                    m2[:], msk[:], axis=mybir.AxisListType.X, op=mybir.AluOpType.max)
                s2 = g_pool.tile([128, E], F32, tag="s2")
                nc.vector.tensor_scalar(
                    s2[:], tt[:], 2.0, m2[:],
                    op0=mybir.AluOpType.mult, op1=mybir.AluOpType.subtract)
                sg = g_pool.tile([128, E], F32, tag="sg")
                nc.scalar.activation(
                    sg[:], s2[:], mybir.ActivationFunctionType.Sigmoid)
                ge = g_pool.tile([128, E], F32, tag="ge")
                nc.vector.tensor_scalar(
                    ge[:], tt[:], m2[:], None, op0=mybir.AluOpType.is_ge)
                nc.vector.tensor_tensor(
                    gates_sb[:, tl, :], ge[:], sg[:], op=mybir.AluOpType.mult)

            # ---- next group's first transpose tile fills PE while gating runs
            if gi + 1 < N_GROUPS:
                xt_next = new_xt(PROC[gi + 1])
                emit_transpose_tile(xt_next, gi + 1, 0)

            # ---- gates -> (e,r)-expanded scale -> midTs ----
            gT_ps = ps_g.tile([E, GROUP], F32, tag="gps")
            for tl in range(TPG):
                nc.tensor.matmul(
                    gT_ps[:, tl * 128:(tl + 1) * 128],
                    gates_sb[:, tl, :],
                    id32_sb[:],
                    is_transpose=True,
                )
            gT_sb = g_pool.tile([E, GROUP], MM2_DT, tag="gT")
            nc.vector.tensor_copy(gT_sb[:], gT_ps[:])

            gexp_ps = ps_g.tile([ER, GROUP], F32, tag="gps")
            nc.tensor.matmul(gexp_ps[:], sel_sb[:], gT_sb[:])
            gexp_sb = g_pool.tile([ER, GROUP], F32, tag="gexp")
            nc.scalar.copy(gexp_sb[:], gexp_ps[:])

            midTs = mid_pool.tile([ER, GROUP], MM2_DT, tag="midTs")
            nc.vector.tensor_tensor(
                midTs[:], mid_ps[0:ER, :], gexp_sb[:], op=mybir.AluOpType.mult)

            # ---- mm2 + stores for THIS group, interleaved with the next
            # group's remaining transpose tile ----
            sq = nc.sync
            emit_mm2_tile(midTs, tok_g, 0, store_q=sq)
            if gi + 1 < N_GROUPS:
                emit_transpose_tile(xt_next, gi + 1, 1)
            emit_mm2_tile(midTs, tok_g, 1, store_q=sq)

            if gi + 1 < N_GROUPS:
                xt_cur = xt_next


_CACHED = {}


def _build_module():
    key = (MM1_DT, MM2_DT, OUT_DT)
    if key in _CACHED:
        return _CACHED[key]
    nc = bacc.Bacc("TRN2", target_bir_lowering=False, debug=False)
    x_d = nc.dram_tensor("x_in", [T_C, D], X_DT, kind="ExternalInput").ap()
    wt_d = nc.dram_tensor(
        "wt_in", [128, D_CHUNKS * M_W], MM1_DT, kind="ExternalInput").ap()
    b_d = nc.dram_tensor("ball_in", [ER, D], MM2_DT, kind="ExternalInput").ap()
    sel_d = nc.dram_tensor("sel_in", [E, ER], MM2_DT, kind="ExternalInput").ap()
    id_d = nc.dram_tensor("id_in", [128, 128], ID_DT, kind="ExternalInput").ap()
    id32_d = nc.dram_tensor(
        "id32_in", [128, 128], F32, kind="ExternalInput").ap()
    out_d = nc.dram_tensor("out", [T_C, D], OUT_DT, kind="ExternalOutput").ap()
    with tile.TileContext(nc) as tc:
        build_kernel(tc, out_d, x_d, wt_d, b_d, sel_d, id_d, id32_d)
    nc.compile()
    _CACHED[key] = nc
    return nc


def _host_weights(router_w, A, B):
    W = np.concatenate([A.reshape(ER, D), router_w], axis=0).astype(np.float32)
    # pack W.T [D, 68] into SBUF partition layout [128, 30*68]:
    # partition p, chunk c, row m  <-  W.T[c*128+p, m]
    WT = np.ascontiguousarray(
        W.T.reshape(D_CHUNKS, 128, M_W).transpose(1, 0, 2).reshape(
            128, D_CHUNKS * M_W))
    B_all = np.ascontiguousarray(
        B.transpose(0, 2, 1).reshape(ER, D)).astype(np.float32)      # [(e,r), d]
    sel = np.zeros((E, ER), np.float32)
    for e in range(E):
        sel[e, e * R:(e + 1) * R] = LORA_SCALE
    import ml_dtypes
    _np_map = {F32: np.float32, F32R: np.float32,
               BF16: ml_dtypes.bfloat16, F16: np.float16}
    ident32 = np.eye(128, dtype=np.float32)
    ident = ident32.astype(_np_map[ID_DT])
    B_all = B_all.astype(_np_map[MM2_DT])
    sel = sel.astype(_np_map[MM2_DT])
    return WT, B_all, sel, ident, ident32


def make_in_maps(x, router_w, A, B):
    flat = np.ascontiguousarray(np.asarray(x, np.float32).reshape(T_FULL, D))
    WT, B_all, sel, ident, ident32 = _host_weights(
        np.asarray(router_w, np.float32),
        np.asarray(A, np.float32),
        np.asarray(B, np.float32))
    in_maps = []
    for i in range(N_CORES):
        in_maps.append({
            "x_in": flat[i * T_C:(i + 1) * T_C],
            "wt_in": WT,
            "ball_in": B_all,
            "sel_in": sel,
            "id_in": ident,
            "id32_in": ident32,
        })
    return in_maps


def kernel(x, router_w, A, B, _results_hook=None):
    from concourse.bass_utils import run_bass_kernel_spmd

    nc = _build_module()
    in_maps = make_in_maps(x, router_w, A, B)
    res = run_bass_kernel_spmd(nc, in_maps, core_ids=list(range(N_CORES)))
    if _results_hook is not None:
        _results_hook(res)
    out = np.concatenate([res.results[i]["out"] for i in range(N_CORES)], axis=0)
    return out.astype(np.float32, copy=False).reshape(B_, S, D)


if __name__ == "__main__":
    rng = np.random.default_rng(0)
    x = rng.standard_normal((B_, S, D), dtype=np.float32)
    rw = (rng.standard_normal((E, D)) * 0.02).astype(np.float32)
    A = (rng.standard_normal((E, R, D)) * 0.02).astype(np.float32)
    Bm = (rng.standard_normal((E, D, R)) * 0.02).astype(np.float32)
    out = kernel(x, rw, A, Bm)
    print("out", out.shape, out.dtype, float(np.abs(out).max()))


# revision 103
# speedup vs baseline: 1.0142x; 1.0077x over previous
"""MoE LoRA delta kernel for Trainium2 (8 NeuronCores, data-parallel over tokens).

Computation (per token t):
    logits = x @ router_w.T                      [T, 4]
    gates  = top2-softmax(logits)                [T, 4]  (exactly 2 nonzero)
    mid    = x @ A_all.T                         [T, 64]   A_all[(e,r), d]
    delta  = (mid * expand(gates) * 4.0) @ B_all [T, D]    B_all[(e,r), d]

Kernel strategy per core (T_c = 1024 tokens, 4 groups of 256):
  - W = concat([A_all, router_w]) -> [68, D]; host packs W.T into the SBUF
    partition layout [128, 30*68] so the weight DMA moves 8KB descriptors.
  - x.T tiles produced on-chip with PE transpose-mode matmuls (fp32, exact),
    evacuated to SBUF as float32r so mm1 runs at 1 cycle/row.
  - mm1 computes [68, 256] = W @ x.T in f32r with fp32 PSUM accumulation;
    rows 64:68 are the router logits.  f32r keeps the logits accurate enough
    that the top-2 expert selection matches the fp32 reference (bf16/fp16 x
    flips 2-10 tokens on the actual dataset, each costing ~0.4 rel err, so
    x must stay 4 bytes; this pins the DMA roofline at ~92us/core).
  - Gating runs in fp32 on the accumulated logits: g_e = 1{t_e >= m2} *
    sigmoid(2*t_e - m2), t = l - max(l).
  - mm2 runs in fp16 (same 1 cycle/row on PE, halves the B_all weight DMA,
    11-bit mantissa); products accumulate in fp32 PSUM.  The output is
    stored as fp16 (halves the store DMA) and the host gather upcasts to
    fp32; measured rel err 6.8e-4 against the 2e-2 budget.
  - Schedule: the next group's transpose tiles are interleaved between this
    group's mm1/gating and mm2 so the PE never waits on the DVE/ACT gating
    chain; PSUM->SBUF evacuations alternate ACT-first/DVE; outputs are
    stored as two half-tile DMAs per 128-token tile.
  - DMA: x loads + weights on the SP HWDGE queue, output stores on the Pool
    SWDGE queue, so the (serialized, 360 GB/s) DMA engines see back-to-back
    transfers from two independent queues: zero idle between the first and
    last transfer in the cost-model timeline.
"""

import os
import sys

for _p in ("/opt/trn_rl_repo", "/root/.axon_site/_ro/trn_rl_repo"):
    if os.path.isdir(_p) and _p not in sys.path:
        sys.path.insert(0, _p)

import numpy as np
from contextlib import ExitStack

import concourse.bass as bass
import concourse.bacc as bacc
import concourse.mybir as mybir
import concourse.tile as tile

N_CORES = 8
B_, S, D = 4, 2048, 3840
T_FULL = B_ * S                 # 8192
T_C = T_FULL // N_CORES         # 1024 tokens per core
E, R = 4, 16
ER = E * R                      # 64
M_W = ER + E                    # 68 = A rows + router rows
LORA_SCALE = 16.0 / np.sqrt(16.0)   # 4.0

GROUP = 256                     # tokens per mm1 group
TPG = GROUP // 128              # token tiles per group (2)
N_GROUPS = T_C // GROUP         # 4
D_CHUNKS = D // 128             # 30
MM2_CHUNKS = [(i * 512, min(512, D - i * 512)) for i in range((D + 511) // 512)]

F32 = mybir.dt.float32
F32R = mybir.dt.float32r

BF16 = mybir.dt.bfloat16
F16 = mybir.dt.float16

# Dtype choices:
#  - mm1 f32r: 1 cyc/row on PE and keeps the router logits at ~19-bit
#    precision so the top-2 expert selection matches the fp32 reference.
#  - mm2 fp16: same PE speed, halves the B_all weight DMA, 11-bit mantissa.
#  - output fp16 on device (halves the store DMA -- the single biggest
#    traffic item after x); the host gather upcasts to fp32.  Quantization
#    adds ~5e-4 rel err against the 2e-2 budget.
_DT_MAP = {"f32": F32, "f32r": F32R, "bf16": BF16, "f16": F16}
MM1_DT = _DT_MAP[os.environ.get("MOE_MM1", "f32r")]
MM2_DT = _DT_MAP[os.environ.get("MOE_MM2", "f16")]
OUT_DT = _DT_MAP[os.environ.get("MOE_OUT", "f16")]
# x streams through the PE transposes as f32r: 1.5 cycles/row vs 2.0 for
# fp32, and numerically identical here (xt is f32r-rounded downstream
# anyway).  The identity must match (walrus rejects mixed 32/16-bit matmul
# inputs, and fp32+anything is broken in HW).
X_DT = _DT_MAP[os.environ.get("MOE_X", "f32r")]
ID_DT = _DT_MAP[os.environ.get("MOE_ID", "f32r")]


def build_kernel(tc: tile.TileContext, out_d, x_d, wt_d, b_d, sel_d, id_d,
                 id32_d):
    nc = tc.nc
    with ExitStack() as ctx:
        const_pool = ctx.enter_context(tc.tile_pool(name="const", bufs=1))
        xin_pool = ctx.enter_context(tc.tile_pool(name="xin", bufs=4))
        xt_pool = ctx.enter_context(tc.tile_pool(name="xt", bufs=2))
        mid_pool = ctx.enter_context(tc.tile_pool(name="mid", bufs=2))
        g_pool = ctx.enter_context(tc.tile_pool(name="gate", bufs=2))
        dout_pool = ctx.enter_context(tc.tile_pool(name="dout", bufs=8))
        ps_tp = ctx.enter_context(
            tc.tile_pool(name="ps_tp", bufs=2, space=bass.MemorySpace.PSUM))
        ps_mm1 = ctx.enter_context(
            tc.tile_pool(name="ps_mm1", bufs=2, space=bass.MemorySpace.PSUM))
        ps_g = ctx.enter_context(
            tc.tile_pool(name="ps_g", bufs=1, space=bass.MemorySpace.PSUM))
        ps_mm2 = ctx.enter_context(
            tc.tile_pool(name="ps_mm2", bufs=3, space=bass.MemorySpace.PSUM))

        PROC = list(range(N_GROUPS))
        PROC_TILES = [g * TPG + tl for g in PROC for tl in range(TPG)]

        # ---- prologue DMAs (SP queue): first x tile first so PE starts early
        x_sb = {}

        def load_x_seq(seq):
            t = PROC_TILES[seq]
            x_sb[t] = xin_pool.tile([128, D], X_DT, tag="xin", name=f"x_t{t}")
            nc.sync.dma_start(x_sb[t][:], x_d[t * 128:(t + 1) * 128, :])

        # Issue ALL loads up front: the xin pool's WAR deps throttle them to
        # the transpose consumption pace, and a parked load always beats a
        # later-arriving store to the DMA engines, so loads finish early and
        # the back half of the bus schedule is pure stores.
        load_x_seq(0)
        id_sb = const_pool.tile([128, 128], ID_DT, tag="ident")
        nc.sync.dma_start(id_sb[:], id_d[:])
        # fp32 identity for the small gating transposes (their data is fp32)
        id32_sb = const_pool.tile([128, 128], F32, tag="ident32")
        nc.sync.dma_start(id32_sb[:], id32_d[:])
        wt_sb = const_pool.tile([128, D_CHUNKS, M_W], MM1_DT, tag="wt")
        nc.sync.dma_start(wt_sb[:], wt_d.rearrange("p (c m) -> p c m", m=M_W))
        load_x_seq(1)
        b_sb = const_pool.tile([ER, D], MM2_DT, tag="ball")
        nc.sync.dma_start(b_sb[:], b_d[:])
        sel_sb = const_pool.tile([E, ER], MM2_DT, tag="sel")
        nc.sync.dma_start(sel_sb[:], sel_d[:])
        for _seq in range(2, N_GROUPS * TPG):
            load_x_seq(_seq)

        cp_engines = [nc.scalar, nc.vector]
        xt_i = 0
        do_i = 0

        TPC = 4   # chunks per transpose-evacuation copy

        def new_xt(g):
            return xt_pool.tile(
                [128, D_CHUNKS, GROUP], MM1_DT, tag="xt", name=f"xt_g{g}")

        def emit_transpose_tile(xt_sb, gi, tl):
            """One 128-token x tile (processing position gi, tile tl) ->
            xt_sb (f32r), PE transposes + DVE/ACT evacuation."""
            nonlocal xt_i
            seq = gi * TPG + tl
            t = PROC_TILES[seq]
            for c0 in range(0, D_CHUNKS, TPC):
                ncc = min(TPC, D_CHUNKS - c0)
                tp_ps = ps_tp.tile([128, TPC, 128], X_DT, tag="tp")
                for cc in range(ncc):
                    c = c0 + cc
                    nc.tensor.transpose(
                        tp_ps[:, cc, :],
                        x_sb[t][:, c * 128:(c + 1) * 128],
                        id_sb[:],
                    )
                eng = cp_engines[xt_i % 2]; xt_i += 1
                dst = xt_sb[:, c0:c0 + ncc, tl * 128:(tl + 1) * 128]
                if eng is nc.vector:
                    eng.tensor_copy(dst, tp_ps[:, 0:ncc, :])
                else:
                    eng.copy(dst, tp_ps[:, 0:ncc, :])

        DSPLIT = 2048   # store each tile as two half-tile DMAs

        def emit_mm2_tile(midTs, tok_g_p, tl, store_q=None):
            # Early groups store on the SP queue: strict FIFO behind every
            # load, so stores can never steal bus slots mid-load-stream
            # (arrival-order arbitration otherwise stretches the last load
            # by ~25us).  Only the final group's stores use the Pool queue.
            nonlocal do_i
            store_q = store_q or nc.sync
            tok0 = tok_g_p + tl * 128
            dout_sb = dout_pool.tile([128, D], OUT_DT, tag="dout")
            for (d0, w) in MM2_CHUNKS:
                mm2_ps = ps_mm2.tile([128, 512], F32, tag="mm2")
                nc.tensor.matmul(
                    mm2_ps[:, 0:w],
                    midTs[:, tl * 128:(tl + 1) * 128],
                    b_sb[:, d0:d0 + w],
                )
                eng = cp_engines[do_i % 2]; do_i += 1
                if eng is nc.vector:
                    eng.tensor_copy(dout_sb[:, d0:d0 + w], mm2_ps[:, 0:w])
                else:
                    eng.copy(dout_sb[:, d0:d0 + w], mm2_ps[:, 0:w])
                if (d0 + w) in (1024, 2048, 3072):
                    store_q.dma_start(
                        out_d[tok0:tok0 + 128, d0 + w - 1024:d0 + w],
                        dout_sb[:, d0 + w - 1024:d0 + w])
            store_q.dma_start(
                out_d[tok0:tok0 + 128, 3072:D], dout_sb[:, 3072:D])

        xt_cur = new_xt(PROC[0])
        for tl in range(TPG):
            emit_transpose_tile(xt_cur, 0, tl)

        for gi, g in enumerate(PROC):
            tok_g = g * GROUP

            # ---- mm1: [68, GROUP] = W @ x.T (f32r, fp32 accumulation) ----
            mid_ps = ps_mm1.tile([M_W, GROUP], F32, tag="mm1")
            for c in range(D_CHUNKS):
                nc.tensor.matmul(
                    mid_ps[:],
                    wt_sb[:, c, :],
                    xt_cur[:, c, :],
                    start=(c == 0),
                    stop=(c == D_CHUNKS - 1),
                )

            # ---- gating (fp32, tokens on partitions) ----
            lg_sb = g_pool.tile([M_W, GROUP], F32, tag="lg")
            nc.vector.tensor_copy(lg_sb[ER:M_W, :], mid_ps[ER:M_W, :])

            logT_ps = ps_g.tile([128, TPG, E], F32, tag="gps")
            for tl in range(TPG):
                nc.tensor.matmul(
                    logT_ps[:, tl, :],
                    lg_sb[ER:M_W, tl * 128:(tl + 1) * 128],
                    id32_sb[ER:M_W, ER:M_W],
                    is_transpose=True,
                )

            gates_sb = g_pool.tile([128, TPG, E], F32, tag="gates")
            for tl in range(TPG):
                L = g_pool.tile([128, E], F32, tag="L")
                nc.vector.tensor_copy(L[:], logT_ps[:, tl, :])
                m1 = g_pool.tile([128, 1], F32, tag="m1")
                nc.vector.tensor_reduce(
                    m1[:], L[:], axis=mybir.AxisListType.X, op=mybir.AluOpType.max)
                tt = g_pool.tile([128, E], F32, tag="tt")
                nc.vector.tensor_scalar(
                    tt[:], L[:], m1[:], None, op0=mybir.AluOpType.subtract)
                z = g_pool.tile([128, E], F32, tag="z")
                nc.vector.tensor_scalar(
                    z[:], tt[:], 0.0, None, op0=mybir.AluOpType.is_equal)
                msk = g_pool.tile([128, E], F32, tag="msk")
                nc.vector.scalar_tensor_tensor(
                    msk[:], z[:], -1e30, tt[:],
                    op0=mybir.AluOpType.mult, op1=mybir.AluOpType.add)
                m2 = g_pool.tile([128, 1], F32, tag="m2")
                nc.vector.tensor_reduce(
                    m2[:], msk[:], axis=mybir.AxisListType.X, op=mybir.AluOpType.max)
                s2 = g_pool.tile([128, E], F32, tag="s2")
                nc.vector.tensor_scalar(
                    s2[:], tt[:], 2.0, m2[:],
                    op0=mybir.AluOpType.mult, op1=mybir.AluOpType.subtract)
                sg = g_pool.tile([128, E], F32, tag="sg")
                nc.scalar.activation(
                    sg[:], s2[:], mybir.ActivationFunctionType.Sigmoid)
                ge = g_pool.tile([128, E], F32, tag="ge")
                nc.vector.tensor_scalar(
                    ge[:], tt[:], m2[:], None, op0=mybir.AluOpType.is_ge)
                nc.vector.tensor_tensor(
                    gates_sb[:, tl, :], ge[:], sg[:], op=mybir.AluOpType.mult)

            # ---- next group's first transpose tile fills PE while gating runs
            if gi + 1 < N_GROUPS:
                xt_next = new_xt(PROC[gi + 1])
                emit_transpose_tile(xt_next, gi + 1, 0)

            # ---- gates -> (e,r)-expanded scale -> midTs ----
            gT_ps = ps_g.tile([E, GROUP], F32, tag="gps")
            for tl in range(TPG):
                nc.tensor.matmul(
                    gT_ps[:, tl * 128:(tl + 1) * 128],
                    gates_sb[:, tl, :],
                    id32_sb[:],
                    is_transpose=True,
                )
            gT_sb = g_pool.tile([E, GROUP], MM2_DT, tag="gT")
            nc.vector.tensor_copy(gT_sb[:], gT_ps[:])

            gexp_ps = ps_g.tile([ER, GROUP], F32, tag="gps")
            nc.tensor.matmul(gexp_ps[:], sel_sb[:], gT_sb[:])
            gexp_sb = g_pool.tile([ER, GROUP], F32, tag="gexp")
            nc.scalar.copy(gexp_sb[:], gexp_ps[:])

            midTs = mid_pool.tile([ER, GROUP], MM2_DT, tag="midTs")
            nc.vector.tensor_tensor(
                midTs[:], mid_ps[0:ER, :], gexp_sb[:], op=mybir.AluOpType.mult)

            # ---- mm2 + stores for THIS group, interleaved with the next
            # group's remaining transpose tile ----
            sq = nc.sync
            emit_mm2_tile(midTs, tok_g, 0, store_q=sq)
            if gi + 1 < N_GROUPS:
                emit_transpose_tile(xt_next, gi + 1, 1)
            emit_mm2_tile(midTs, tok_g, 1, store_q=sq)

            if gi + 1 < N_GROUPS:
                xt_cur = xt_next


_CACHED = {}


def _build_module():
    key = (MM1_DT, MM2_DT, OUT_DT)
    if key in _CACHED:
        return _CACHED[key]
    nc = bacc.Bacc("TRN2", target_bir_lowering=False, debug=False)
    x_d = nc.dram_tensor("x_in", [T_C, D], X_DT, kind="ExternalInput").ap()
    wt_d = nc.dram_tensor(
        "wt_in", [128, D_CHUNKS * M_W], MM1_DT, kind="ExternalInput").ap()
    b_d = nc.dram_tensor("ball_in", [ER, D], MM2_DT, kind="ExternalInput").ap()
    sel_d = nc.dram_tensor("sel_in", [E, ER], MM2_DT, kind="ExternalInput").ap()
    id_d = nc.dram_tensor("id_in", [128, 128], ID_DT, kind="ExternalInput").ap()
    id32_d = nc.dram_tensor(
        "id32_in", [128, 128], F32, kind="ExternalInput").ap()
    out_d = nc.dram_tensor("out", [T_C, D], OUT_DT, kind="ExternalOutput").ap()
    with tile.TileContext(nc) as tc:
        build_kernel(tc, out_d, x_d, wt_d, b_d, sel_d, id_d, id32_d)
    nc.compile()
    _CACHED[key] = nc
    return nc


def _host_weights(router_w, A, B):
    W = np.concatenate([A.reshape(ER, D), router_w], axis=0).astype(np.float32)
    # pack W.T [D, 68] into SBUF partition layout [128, 30*68]:
    # partition p, chunk c, row m  <-  W.T[c*128+p, m]
    WT = np.ascontiguousarray(
        W.T.reshape(D_CHUNKS, 128, M_W).transpose(1, 0, 2).reshape(
            128, D_CHUNKS * M_W))
    B_all = np.ascontiguousarray(
        B.transpose(0, 2, 1).reshape(ER, D)).astype(np.float32)      # [(e,r), d]
    sel = np.zeros((E, ER), np.float32)
    for e in range(E):
        sel[e, e * R:(e + 1) * R] = LORA_SCALE
    import ml_dtypes
    _np_map = {F32: np.float32, F32R: np.float32,
               BF16: ml_dtypes.bfloat16, F16: np.float16}
    ident32 = np.eye(128, dtype=np.float32)
    ident = ident32.astype(_np_map[ID_DT])
    B_all = B_all.astype(_np_map[MM2_DT])
    sel = sel.astype(_np_map[MM2_DT])
    return WT, B_all, sel, ident, ident32


def make_in_maps(x, router_w, A, B):
    flat = np.ascontiguousarray(np.asarray(x, np.float32).reshape(T_FULL, D))
    WT, B_all, sel, ident, ident32 = _host_weights(
        np.asarray(router_w, np.float32),
        np.asarray(A, np.float32),
        np.asarray(B, np.float32))
    in_maps = []
    for i in range(N_CORES):
        in_maps.append({
            "x_in": flat[i * T_C:(i + 1) * T_C],
            "wt_in": WT,
            "ball_in": B_all,
            "sel_in": sel,
            "id_in": ident,
            "id32_in": ident32,
        })
    return in_maps


def kernel(x, router_w, A, B, _results_hook=None):
    from concourse.bass_utils import run_bass_kernel_spmd

    nc = _build_module()
    in_maps = make_in_maps(x, router_w, A, B)
    res = run_bass_kernel_spmd(nc, in_maps, core_ids=list(range(N_CORES)))
    if _results_hook is not None:
        _results_hook(res)
    out = np.concatenate([res.results[i]["out"] for i in range(N_CORES)], axis=0)
    return out.astype(np.float32, copy=False).reshape(B_, S, D)


if __name__ == "__main__":
    rng = np.random.default_rng(0)
    x = rng.standard_normal((B_, S, D), dtype=np.float32)
    rw = (rng.standard_normal((E, D)) * 0.02).astype(np.float32)
    A = (rng.standard_normal((E, R, D)) * 0.02).astype(np.float32)
    Bm = (rng.standard_normal((E, D, R)) * 0.02).astype(np.float32)
    out = kernel(x, rw, A, Bm)
    print("out", out.shape, out.dtype, float(np.abs(out).max()))


# revision 110
# speedup vs baseline: 1.0333x; 1.0188x over previous
"""MoE LoRA delta kernel for Trainium2 (8 NeuronCores, data-parallel over tokens).

Computation (per token t):
    logits = x @ router_w.T                      [T, 4]
    gates  = top2-softmax(logits)                [T, 4]  (exactly 2 nonzero)
    mid    = x @ A_all.T                         [T, 64]   A_all[(e,r), d]
    delta  = (mid * expand(gates) * 4.0) @ B_all [T, D]    B_all[(e,r), d]

Kernel strategy per core (T_c = 1024 tokens, 4 groups of 256):
  - W = concat([A_all, router_w]) -> [68, D]; host packs W.T into the SBUF
    partition layout [128, 30*68] so the weight DMA moves 8KB descriptors.
  - x.T tiles produced on-chip with PE transpose-mode matmuls (fp32, exact),
    evacuated to SBUF as float32r so mm1 runs at 1 cycle/row.
  - mm1 computes [68, 256] = W @ x.T in f32r with fp32 PSUM accumulation;
    rows 64:68 are the router logits.  f32r keeps the logits accurate enough
    that the top-2 expert selection matches the fp32 reference (bf16/fp16 x
    flips 2-10 tokens on the actual dataset, each costing ~0.4 rel err, so
    x must stay 4 bytes; this pins the DMA roofline at ~92us/core).
  - Gating runs in fp32 on the accumulated logits: g_e = 1{t_e >= m2} *
    sigmoid(2*t_e - m2), t = l - max(l).
  - mm2 runs in fp16 (same 1 cycle/row on PE, halves the B_all weight DMA,
    11-bit mantissa); products accumulate in fp32 PSUM.  The output is
    stored as fp16 (halves the store DMA) and the host gather upcasts to
    fp32; measured rel err 6.8e-4 against the 2e-2 budget.
  - Schedule: the next group's transpose tiles are interleaved between this
    group's mm1/gating and mm2 so the PE never waits on the DVE/ACT gating
    chain; PSUM->SBUF evacuations alternate ACT-first/DVE; outputs are
    stored as two half-tile DMAs per 128-token tile.
  - DMA: x loads + weights on the SP HWDGE queue, output stores on the Pool
    SWDGE queue, so the (serialized, 360 GB/s) DMA engines see back-to-back
    transfers from two independent queues: zero idle between the first and
    last transfer in the cost-model timeline.
"""

import os
import sys

for _p in ("/opt/trn_rl_repo", "/root/.axon_site/_ro/trn_rl_repo"):
    if os.path.isdir(_p) and _p not in sys.path:
        sys.path.insert(0, _p)

import numpy as np
from contextlib import ExitStack

import concourse.bass as bass
import concourse.bacc as bacc
import concourse.mybir as mybir
import concourse.tile as tile

N_CORES = 8
B_, S, D = 4, 2048, 3840
T_FULL = B_ * S                 # 8192
T_C = T_FULL // N_CORES         # 1024 tokens per core
E, R = 4, 16
ER = E * R                      # 64
M_W = ER + E                    # 68 = A rows + router rows
LORA_SCALE = 16.0 / np.sqrt(16.0)   # 4.0

GROUP = 256                     # tokens per mm1 group
TPG = GROUP // 128              # token tiles per group (2)
N_GROUPS = T_C // GROUP         # 4
D_CHUNKS = D // 128             # 30
MM2_CHUNKS = [(i * 512, min(512, D - i * 512)) for i in range((D + 511) // 512)]

F32 = mybir.dt.float32
F32R = mybir.dt.float32r

BF16 = mybir.dt.bfloat16
F16 = mybir.dt.float16

# Dtype choices:
#  - mm1 f32r: 1 cyc/row on PE and keeps the router logits at ~19-bit
#    precision so the top-2 expert selection matches the fp32 reference.
#  - mm2 fp16: same PE speed, halves the B_all weight DMA, 11-bit mantissa.
#  - output fp16 on device (halves the store DMA -- the single biggest
#    traffic item after x); the host gather upcasts to fp32.  Quantization
#    adds ~5e-4 rel err against the 2e-2 budget.
_DT_MAP = {"f32": F32, "f32r": F32R, "bf16": BF16, "f16": F16}
MM1_DT = _DT_MAP[os.environ.get("MOE_MM1", "f32r")]
MM2_DT = _DT_MAP[os.environ.get("MOE_MM2", "f16")]
OUT_DT = _DT_MAP[os.environ.get("MOE_OUT", "f16")]
# x streams through the PE transposes as f32r: 1.5 cycles/row vs 2.0 for
# fp32, and numerically identical here (xt is f32r-rounded downstream
# anyway).  The identity must match (walrus rejects mixed 32/16-bit matmul
# inputs, and fp32+anything is broken in HW).
X_DT = _DT_MAP[os.environ.get("MOE_X", "f32r")]
ID_DT = _DT_MAP[os.environ.get("MOE_ID", "f32r")]


def build_kernel(tc: tile.TileContext, out_d, x_d, wt_d, b_d, sel_d, id_d,
                 id32_d):
    nc = tc.nc
    with ExitStack() as ctx:
        const_pool = ctx.enter_context(tc.tile_pool(name="const", bufs=1))
        xin_pool = ctx.enter_context(tc.tile_pool(name="xin", bufs=4))
        xt_pool = ctx.enter_context(tc.tile_pool(name="xt", bufs=2))
        mid_pool = ctx.enter_context(tc.tile_pool(name="mid", bufs=2))
        g_pool = ctx.enter_context(tc.tile_pool(name="gate", bufs=2))
        dout_pool = ctx.enter_context(tc.tile_pool(name="dout", bufs=8))
        ps_tp = ctx.enter_context(
            tc.tile_pool(name="ps_tp", bufs=2, space=bass.MemorySpace.PSUM))
        ps_mm1 = ctx.enter_context(
            tc.tile_pool(name="ps_mm1", bufs=2, space=bass.MemorySpace.PSUM))
        ps_g = ctx.enter_context(
            tc.tile_pool(name="ps_g", bufs=1, space=bass.MemorySpace.PSUM))
        ps_mm2 = ctx.enter_context(
            tc.tile_pool(name="ps_mm2", bufs=3, space=bass.MemorySpace.PSUM))

        PROC = list(range(N_GROUPS))
        PROC_TILES = [g * TPG + tl for g in PROC for tl in range(TPG)]

        # ---- prologue DMAs (SP queue): first x tile first so PE starts early
        x_sb = {}

        def load_x_seq(seq):
            t = PROC_TILES[seq]
            x_sb[t] = xin_pool.tile([128, D], X_DT, tag="xin", name=f"x_t{t}")
            q = D // 8
            for k in range(8):
                nc.sync.dma_start(
                    x_sb[t][:, k * q:(k + 1) * q],
                    x_d[t * 128:(t + 1) * 128, k * q:(k + 1) * q])

        # Issue ALL loads up front: the xin pool's WAR deps throttle them to
        # the transpose consumption pace, and a parked load always beats a
        # later-arriving store to the DMA engines, so loads finish early and
        # the back half of the bus schedule is pure stores.
        load_x_seq(0)
        id_sb = const_pool.tile([128, 128], ID_DT, tag="ident")
        nc.sync.dma_start(id_sb[:], id_d[:])
        # fp32 identity for the small gating transposes (their data is fp32)
        id32_sb = const_pool.tile([128, 128], F32, tag="ident32")
        nc.sync.dma_start(id32_sb[:], id32_d[:])
        wt_sb = const_pool.tile([128, D_CHUNKS, M_W], MM1_DT, tag="wt")
        nc.sync.dma_start(wt_sb[:], wt_d.rearrange("p (c m) -> p c m", m=M_W))
        load_x_seq(1)
        b_sb = const_pool.tile([ER, D], MM2_DT, tag="ball")
        nc.sync.dma_start(b_sb[:], b_d[:])
        sel_sb = const_pool.tile([E, ER], MM2_DT, tag="sel")
        nc.sync.dma_start(sel_sb[:], sel_d[:])
        for _seq in range(2, N_GROUPS * TPG):
            load_x_seq(_seq)

        cp_engines = [nc.scalar, nc.vector]
        xt_i = 0
        do_i = 0

        TPC = 4   # chunks per transpose-evacuation copy

        def new_xt(g):
            return xt_pool.tile(
                [128, D_CHUNKS, GROUP], MM1_DT, tag="xt", name=f"xt_g{g}")

        def emit_transpose_tile(xt_sb, gi, tl):
            """One 128-token x tile (processing position gi, tile tl) ->
            xt_sb (f32r), PE transposes + DVE/ACT evacuation."""
            nonlocal xt_i
            seq = gi * TPG + tl
            t = PROC_TILES[seq]
            for c0 in range(0, D_CHUNKS, TPC):
                ncc = min(TPC, D_CHUNKS - c0)
                tp_ps = ps_tp.tile([128, TPC, 128], X_DT, tag="tp")
                for cc in range(ncc):
                    c = c0 + cc
                    nc.tensor.transpose(
                        tp_ps[:, cc, :],
                        x_sb[t][:, c * 128:(c + 1) * 128],
                        id_sb[:],
                    )
                eng = cp_engines[xt_i % 2]; xt_i += 1
                dst = xt_sb[:, c0:c0 + ncc, tl * 128:(tl + 1) * 128]
                if eng is nc.vector:
                    eng.tensor_copy(dst, tp_ps[:, 0:ncc, :])
                else:
                    eng.copy(dst, tp_ps[:, 0:ncc, :])

        DSPLIT = 2048   # store each tile as two half-tile DMAs

        def emit_mm2_tile(midTs, tok_g_p, tl, store_q=None):
            # Early groups store on the SP queue: strict FIFO behind every
            # load, so stores can never steal bus slots mid-load-stream
            # (arrival-order arbitration otherwise stretches the last load
            # by ~25us).  Only the final group's stores use the Pool queue.
            nonlocal do_i
            store_q = store_q or nc.sync
            tok0 = tok_g_p + tl * 128
            dout_sb = dout_pool.tile([128, D], OUT_DT, tag="dout")
            for (d0, w) in MM2_CHUNKS:
                mm2_ps = ps_mm2.tile([128, 512], F32, tag="mm2")
                nc.tensor.matmul(
                    mm2_ps[:, 0:w],
                    midTs[:, tl * 128:(tl + 1) * 128],
                    b_sb[:, d0:d0 + w],
                )
                eng = cp_engines[do_i % 2]; do_i += 1
                if eng is nc.vector:
                    eng.tensor_copy(dout_sb[:, d0:d0 + w], mm2_ps[:, 0:w])
                else:
                    eng.copy(dout_sb[:, d0:d0 + w], mm2_ps[:, 0:w])
                if (d0 + w) in (1024, 2048, 3072):
                    store_q.dma_start(
                        out_d[tok0:tok0 + 128, d0 + w - 1024:d0 + w],
                        dout_sb[:, d0 + w - 1024:d0 + w])
            store_q.dma_start(
                out_d[tok0:tok0 + 128, 3072:D], dout_sb[:, 3072:D])

        xt_cur = new_xt(PROC[0])
        for tl in range(TPG):
            emit_transpose_tile(xt_cur, 0, tl)

        for gi, g in enumerate(PROC):
            tok_g = g * GROUP

            # ---- mm1: [68, GROUP] = W @ x.T (f32r, fp32 accumulation) ----
            mid_ps = ps_mm1.tile([M_W, GROUP], F32, tag="mm1")
            for c in range(D_CHUNKS):
                nc.tensor.matmul(
                    mid_ps[:],
                    wt_sb[:, c, :],
                    xt_cur[:, c, :],
                    start=(c == 0),
                    stop=(c == D_CHUNKS - 1),
                )

            # ---- gating (fp32, tokens on partitions) ----
            lg_sb = g_pool.tile([M_W, GROUP], F32, tag="lg")
            nc.vector.tensor_copy(lg_sb[ER:M_W, :], mid_ps[ER:M_W, :])

            logT_ps = ps_g.tile([128, TPG, E], F32, tag="gps")
            for tl in range(TPG):
                nc.tensor.matmul(
                    logT_ps[:, tl, :],
                    lg_sb[ER:M_W, tl * 128:(tl + 1) * 128],
                    id32_sb[ER:M_W, ER:M_W],
                    is_transpose=True,
                )

            gates_sb = g_pool.tile([128, TPG, E], F32, tag="gates")
            for tl in range(TPG):
                L = g_pool.tile([128, E], F32, tag="L")
                nc.vector.tensor_copy(L[:], logT_ps[:, tl, :])
                m1 = g_pool.tile([128, 1], F32, tag="m1")
                nc.vector.tensor_reduce(
                    m1[:], L[:], axis=mybir.AxisListType.X, op=mybir.AluOpType.max)
                tt = g_pool.tile([128, E], F32, tag="tt")
                nc.vector.tensor_scalar(
                    tt[:], L[:], m1[:], None, op0=mybir.AluOpType.subtract)
                z = g_pool.tile([128, E], F32, tag="z")
                nc.vector.tensor_scalar(
                    z[:], tt[:], 0.0, None, op0=mybir.AluOpType.is_equal)
                msk = g_pool.tile([128, E], F32, tag="msk")
                nc.vector.scalar_tensor_tensor(
                    msk[:], z[:], -1e30, tt[:],
                    op0=mybir.AluOpType.mult, op1=mybir.AluOpType.add)
                m2 = g_pool.tile([128, 1], F32, tag="m2")
                nc.vector.tensor_reduce(
                    m2[:], msk[:], axis=mybir.AxisListType.X, op=mybir.AluOpType.max)
                s2 = g_pool.tile([128, E], F32, tag="s2")
                nc.vector.tensor_scalar(
                    s2[:], tt[:], 2.0, m2[:],
                    op0=mybir.AluOpType.mult, op1=mybir.AluOpType.subtract)
                sg = g_pool.tile([128, E], F32, tag="sg")
                nc.scalar.activation(
                    sg[:], s2[:], mybir.ActivationFunctionType.Sigmoid)
                ge = g_pool.tile([128, E], F32, tag="ge")
                nc.vector.tensor_scalar(
                    ge[:], tt[:], m2[:], None, op0=mybir.AluOpType.is_ge)
                nc.vector.tensor_tensor(
                    gates_sb[:, tl, :], ge[:], sg[:], op=mybir.AluOpType.mult)

            # ---- next group's first transpose tile fills PE while gating runs
            if gi + 1 < N_GROUPS:
                xt_next = new_xt(PROC[gi + 1])
                emit_transpose_tile(xt_next, gi + 1, 0)

            # ---- gates -> (e,r)-expanded scale -> midTs ----
            gT_ps = ps_g.tile([E, GROUP], F32, tag="gps")
            for tl in range(TPG):
                nc.tensor.matmul(
                    gT_ps[:, tl * 128:(tl + 1) * 128],
                    gates_sb[:, tl, :],
                    id32_sb[:],
                    is_transpose=True,
                )
            gT_sb = g_pool.tile([E, GROUP], MM2_DT, tag="gT")
            nc.vector.tensor_copy(gT_sb[:], gT_ps[:])

            gexp_ps = ps_g.tile([ER, GROUP], F32, tag="gps")
            nc.tensor.matmul(gexp_ps[:], sel_sb[:], gT_sb[:])
            gexp_sb = g_pool.tile([ER, GROUP], F32, tag="gexp")
            nc.scalar.copy(gexp_sb[:], gexp_ps[:])

            midTs = mid_pool.tile([ER, GROUP], MM2_DT, tag="midTs")
            nc.vector.tensor_tensor(
                midTs[:], mid_ps[0:ER, :], gexp_sb[:], op=mybir.AluOpType.mult)

            # ---- mm2 + stores for THIS group, interleaved with the next
            # group's remaining transpose tile ----
            sq = nc.sync
            emit_mm2_tile(midTs, tok_g, 0, store_q=sq)
            if gi + 1 < N_GROUPS:
                emit_transpose_tile(xt_next, gi + 1, 1)
            emit_mm2_tile(midTs, tok_g, 1, store_q=sq)

            if gi + 1 < N_GROUPS:
                xt_cur = xt_next


_CACHED = {}


def _build_module():
    key = (MM1_DT, MM2_DT, OUT_DT)
    if key in _CACHED:
        return _CACHED[key]
    nc = bacc.Bacc("TRN2", target_bir_lowering=False, debug=False)
    x_d = nc.dram_tensor("x_in", [T_C, D], X_DT, kind="ExternalInput").ap()
    wt_d = nc.dram_tensor(
        "wt_in", [128, D_CHUNKS * M_W], MM1_DT, kind="ExternalInput").ap()
    b_d = nc.dram_tensor("ball_in", [ER, D], MM2_DT, kind="ExternalInput").ap()
    sel_d = nc.dram_tensor("sel_in", [E, ER], MM2_DT, kind="ExternalInput").ap()
    id_d = nc.dram_tensor("id_in", [128, 128], ID_DT, kind="ExternalInput").ap()
    id32_d = nc.dram_tensor(
        "id32_in", [128, 128], F32, kind="ExternalInput").ap()
    out_d = nc.dram_tensor("out", [T_C, D], OUT_DT, kind="ExternalOutput").ap()
    with tile.TileContext(nc) as tc:
        build_kernel(tc, out_d, x_d, wt_d, b_d, sel_d, id_d, id32_d)
    nc.compile()
    _CACHED[key] = nc
    return nc


def _host_weights(router_w, A, B):
    W = np.concatenate([A.reshape(ER, D), router_w], axis=0).astype(np.float32)
    # pack W.T [D, 68] into SBUF partition layout [128, 30*68]:
    # partition p, chunk c, row m  <-  W.T[c*128+p, m]
    WT = np.ascontiguousarray(
        W.T.reshape(D_CHUNKS, 128, M_W).transpose(1, 0, 2).reshape(
            128, D_CHUNKS * M_W))
    B_all = np.ascontiguousarray(
        B.transpose(0, 2, 1).reshape(ER, D)).astype(np.float32)      # [(e,r), d]
    sel = np.zeros((E, ER), np.float32)
    for e in range(E):
        sel[e, e * R:(e + 1) * R] = LORA_SCALE
    import ml_dtypes
    _np_map = {F32: np.float32, F32R: np.float32,
               BF16: ml_dtypes.bfloat16, F16: np.float16}
    ident32 = np.eye(128, dtype=np.float32)
    ident = ident32.astype(_np_map[ID_DT])
    B_all = B_all.astype(_np_map[MM2_DT])
    sel = sel.astype(_np_map[MM2_DT])
    return WT, B_all, sel, ident, ident32


def make_in_maps(x, router_w, A, B):
    flat = np.ascontiguousarray(np.asarray(x, np.float32).reshape(T_FULL, D))
    WT, B_all, sel, ident, ident32 = _host_weights(
        np.asarray(router_w, np.float32),
        np.asarray(A, np.float32),
        np.asarray(B, np.float32))
    in_maps = []
    for i in range(N_CORES):
        in_maps.append({
            "x_in": flat[i * T_C:(i + 1) * T_C],
            "wt_in": WT,
            "ball_in": B_all,
            "sel_in": sel,
            "id_in": ident,
            "id32_in": ident32,
        })
    return in_maps


def kernel(x, router_w, A, B, _results_hook=None):
    from concourse.bass_utils import run_bass_kernel_spmd

    nc = _build_module()
    in_maps = make_in_maps(x, router_w, A, B)
    res = run_bass_kernel_spmd(nc, in_maps, core_ids=list(range(N_CORES)))
    if _results_hook is not None:
        _results_hook(res)
    out = np.concatenate([res.results[i]["out"] for i in range(N_CORES)], axis=0)
    return out.astype(np.float32, copy=False).reshape(B_, S, D)


if __name__ == "__main__":
    rng = np.random.default_rng(0)
    x = rng.standard_normal((B_, S, D), dtype=np.float32)
    rw = (rng.standard_normal((E, D)) * 0.02).astype(np.float32)
    A = (rng.standard_normal((E, R, D)) * 0.02).astype(np.float32)
    Bm = (rng.standard_normal((E, D, R)) * 0.02).astype(np.float32)
    out = kernel(x, rw, A, Bm)
    print("out", out.shape, out.dtype, float(np.abs(out).max()))


# revision 112
# speedup vs baseline: 1.0386x; 1.0052x over previous
"""MoE LoRA delta kernel for Trainium2 (8 NeuronCores, data-parallel over tokens).

Computation (per token t):
    logits = x @ router_w.T                      [T, 4]
    gates  = top2-softmax(logits)                [T, 4]  (exactly 2 nonzero)
    mid    = x @ A_all.T                         [T, 64]   A_all[(e,r), d]
    delta  = (mid * expand(gates) * 4.0) @ B_all [T, D]    B_all[(e,r), d]

Kernel strategy per core (T_c = 1024 tokens, 4 groups of 256):
  - W = concat([A_all, router_w]) -> [68, D]; host packs W.T into the SBUF
    partition layout [128, 30*68] so the weight DMA moves 8KB descriptors.
  - x.T tiles produced on-chip with PE transpose-mode matmuls (fp32, exact),
    evacuated to SBUF as float32r so mm1 runs at 1 cycle/row.
  - mm1 computes [68, 256] = W @ x.T in f32r with fp32 PSUM accumulation;
    rows 64:68 are the router logits.  f32r keeps the logits accurate enough
    that the top-2 expert selection matches the fp32 reference (bf16/fp16 x
    flips 2-10 tokens on the actual dataset, each costing ~0.4 rel err, so
    x must stay 4 bytes; this pins the DMA roofline at ~92us/core).
  - Gating runs in fp32 on the accumulated logits: g_e = 1{t_e >= m2} *
    sigmoid(2*t_e - m2), t = l - max(l).
  - mm2 runs in fp16 (same 1 cycle/row on PE, halves the B_all weight DMA,
    11-bit mantissa); products accumulate in fp32 PSUM.  The output is
    stored as fp16 (halves the store DMA) and the host gather upcasts to
    fp32; measured rel err 6.8e-4 against the 2e-2 budget.
  - Schedule: the next group's transpose tiles are interleaved between this
    group's mm1/gating and mm2 so the PE never waits on the DVE/ACT gating
    chain; PSUM->SBUF evacuations alternate ACT-first/DVE; outputs are
    stored as two half-tile DMAs per 128-token tile.
  - DMA: x loads + weights on the SP HWDGE queue, output stores on the Pool
    SWDGE queue, so the (serialized, 360 GB/s) DMA engines see back-to-back
    transfers from two independent queues: zero idle between the first and
    last transfer in the cost-model timeline.
"""

import os
import sys

for _p in ("/opt/trn_rl_repo", "/root/.axon_site/_ro/trn_rl_repo"):
    if os.path.isdir(_p) and _p not in sys.path:
        sys.path.insert(0, _p)

import numpy as np
from contextlib import ExitStack

import concourse.bass as bass
import concourse.bacc as bacc
import concourse.mybir as mybir
import concourse.tile as tile

N_CORES = 8
B_, S, D = 4, 2048, 3840
T_FULL = B_ * S                 # 8192
T_C = T_FULL // N_CORES         # 1024 tokens per core
E, R = 4, 16
ER = E * R                      # 64
M_W = ER + E                    # 68 = A rows + router rows
LORA_SCALE = 16.0 / np.sqrt(16.0)   # 4.0

GROUP = 256                     # tokens per mm1 group
TPG = GROUP // 128              # token tiles per group (2)
N_GROUPS = T_C // GROUP         # 4
D_CHUNKS = D // 128             # 30
MM2_CHUNKS = [(i * 512, min(512, D - i * 512)) for i in range((D + 511) // 512)]

F32 = mybir.dt.float32
F32R = mybir.dt.float32r

BF16 = mybir.dt.bfloat16
F16 = mybir.dt.float16

# Dtype choices:
#  - mm1 f32r: 1 cyc/row on PE and keeps the router logits at ~19-bit
#    precision so the top-2 expert selection matches the fp32 reference.
#  - mm2 fp16: same PE speed, halves the B_all weight DMA, 11-bit mantissa.
#  - output fp16 on device (halves the store DMA -- the single biggest
#    traffic item after x); the host gather upcasts to fp32.  Quantization
#    adds ~5e-4 rel err against the 2e-2 budget.
_DT_MAP = {"f32": F32, "f32r": F32R, "bf16": BF16, "f16": F16}
MM1_DT = _DT_MAP[os.environ.get("MOE_MM1", "f32r")]
MM2_DT = _DT_MAP[os.environ.get("MOE_MM2", "f16")]
OUT_DT = _DT_MAP[os.environ.get("MOE_OUT", "f16")]
# x streams through the PE transposes as f32r: 1.5 cycles/row vs 2.0 for
# fp32, and numerically identical here (xt is f32r-rounded downstream
# anyway).  The identity must match (walrus rejects mixed 32/16-bit matmul
# inputs, and fp32+anything is broken in HW).
X_DT = _DT_MAP[os.environ.get("MOE_X", "f32r")]
ID_DT = _DT_MAP[os.environ.get("MOE_ID", "f32r")]


def build_kernel(tc: tile.TileContext, out_d, x_d, wt_d, b_d, sel_d, id_d,
                 id32_d):
    nc = tc.nc
    with ExitStack() as ctx:
        const_pool = ctx.enter_context(tc.tile_pool(name="const", bufs=1))
        xin_pool = ctx.enter_context(tc.tile_pool(name="xin", bufs=4))
        xt_pool = ctx.enter_context(tc.tile_pool(name="xt", bufs=2))
        mid_pool = ctx.enter_context(tc.tile_pool(name="mid", bufs=2))
        g_pool = ctx.enter_context(tc.tile_pool(name="gate", bufs=2))
        dout_pool = ctx.enter_context(tc.tile_pool(name="dout", bufs=8))
        ps_tp = ctx.enter_context(
            tc.tile_pool(name="ps_tp", bufs=2, space=bass.MemorySpace.PSUM))
        ps_mm1 = ctx.enter_context(
            tc.tile_pool(name="ps_mm1", bufs=2, space=bass.MemorySpace.PSUM))
        ps_g = ctx.enter_context(
            tc.tile_pool(name="ps_g", bufs=1, space=bass.MemorySpace.PSUM))
        ps_mm2 = ctx.enter_context(
            tc.tile_pool(name="ps_mm2", bufs=3, space=bass.MemorySpace.PSUM))

        PROC = list(range(N_GROUPS))
        PROC_TILES = [g * TPG + tl for g in PROC for tl in range(TPG)]

        # ---- prologue DMAs (SP queue): first x tile first so PE starts early
        x_sb = {}

        def load_x_seq(seq):
            t = PROC_TILES[seq]
            x_sb[t] = xin_pool.tile([128, D], X_DT, tag="xin", name=f"x_t{t}")
            for k0 in range(0, D, 512):
                w = min(512, D - k0)
                nc.sync.dma_start(
                    x_sb[t][:, k0:k0 + w],
                    x_d[t * 128:(t + 1) * 128, k0:k0 + w])

        # Issue ALL loads up front: the xin pool's WAR deps throttle them to
        # the transpose consumption pace, and a parked load always beats a
        # later-arriving store to the DMA engines, so loads finish early and
        # the back half of the bus schedule is pure stores.
        load_x_seq(0)
        id_sb = const_pool.tile([128, 128], ID_DT, tag="ident")
        nc.sync.dma_start(id_sb[:], id_d[:])
        # fp32 identity for the small gating transposes (their data is fp32)
        id32_sb = const_pool.tile([128, 128], F32, tag="ident32")
        nc.sync.dma_start(id32_sb[:], id32_d[:])
        wt_sb = const_pool.tile([128, D_CHUNKS, M_W], MM1_DT, tag="wt")
        nc.sync.dma_start(wt_sb[:], wt_d.rearrange("p (c m) -> p c m", m=M_W))
        load_x_seq(1)
        b_sb = const_pool.tile([ER, D], MM2_DT, tag="ball")
        nc.sync.dma_start(b_sb[:], b_d[:])
        sel_sb = const_pool.tile([E, ER], MM2_DT, tag="sel")
        nc.sync.dma_start(sel_sb[:], sel_d[:])
        for _seq in range(2, N_GROUPS * TPG):
            load_x_seq(_seq)

        cp_engines = [nc.scalar, nc.vector]
        xt_i = 0
        do_i = 0

        TPC = 4   # chunks per transpose-evacuation copy

        def new_xt(g):
            return xt_pool.tile(
                [128, D_CHUNKS, GROUP], MM1_DT, tag="xt", name=f"xt_g{g}")

        def emit_transpose_tile(xt_sb, gi, tl):
            """One 128-token x tile (processing position gi, tile tl) ->
            xt_sb (f32r), PE transposes + DVE/ACT evacuation."""
            nonlocal xt_i
            seq = gi * TPG + tl
            t = PROC_TILES[seq]
            for c0 in range(0, D_CHUNKS, TPC):
                ncc = min(TPC, D_CHUNKS - c0)
                tp_ps = ps_tp.tile([128, TPC, 128], X_DT, tag="tp")
                for cc in range(ncc):
                    c = c0 + cc
                    nc.tensor.transpose(
                        tp_ps[:, cc, :],
                        x_sb[t][:, c * 128:(c + 1) * 128],
                        id_sb[:],
                    )
                eng = cp_engines[xt_i % 2]; xt_i += 1
                dst = xt_sb[:, c0:c0 + ncc, tl * 128:(tl + 1) * 128]
                if eng is nc.vector:
                    eng.tensor_copy(dst, tp_ps[:, 0:ncc, :])
                else:
                    eng.copy(dst, tp_ps[:, 0:ncc, :])

        DSPLIT = 2048   # store each tile as two half-tile DMAs

        def emit_mm2_tile(midTs, tok_g_p, tl, store_q=None):
            # Early groups store on the SP queue: strict FIFO behind every
            # load, so stores can never steal bus slots mid-load-stream
            # (arrival-order arbitration otherwise stretches the last load
            # by ~25us).  Only the final group's stores use the Pool queue.
            nonlocal do_i
            store_q = store_q or nc.sync
            tok0 = tok_g_p + tl * 128
            dout_sb = dout_pool.tile([128, D], OUT_DT, tag="dout")
            for (d0, w) in MM2_CHUNKS:
                mm2_ps = ps_mm2.tile([128, 512], F32, tag="mm2")
                nc.tensor.matmul(
                    mm2_ps[:, 0:w],
                    midTs[:, tl * 128:(tl + 1) * 128],
                    b_sb[:, d0:d0 + w],
                )
                eng = cp_engines[do_i % 2]; do_i += 1
                if eng is nc.vector:
                    eng.tensor_copy(dout_sb[:, d0:d0 + w], mm2_ps[:, 0:w])
                else:
                    eng.copy(dout_sb[:, d0:d0 + w], mm2_ps[:, 0:w])
                if (d0 + w) in (1024, 2048, 3072):
                    store_q.dma_start(
                        out_d[tok0:tok0 + 128, d0 + w - 1024:d0 + w],
                        dout_sb[:, d0 + w - 1024:d0 + w])
            store_q.dma_start(
                out_d[tok0:tok0 + 128, 3072:D], dout_sb[:, 3072:D])

        xt_cur = new_xt(PROC[0])
        for tl in range(TPG):
            emit_transpose_tile(xt_cur, 0, tl)

        for gi, g in enumerate(PROC):
            tok_g = g * GROUP

            # ---- mm1: [68, GROUP] = W @ x.T (f32r, fp32 accumulation) ----
            mid_ps = ps_mm1.tile([M_W, GROUP], F32, tag="mm1")
            for c in range(D_CHUNKS):
                nc.tensor.matmul(
                    mid_ps[:],
                    wt_sb[:, c, :],
                    xt_cur[:, c, :],
                    start=(c == 0),
                    stop=(c == D_CHUNKS - 1),
                )

            # ---- gating (fp32, tokens on partitions) ----
            lg_sb = g_pool.tile([M_W, GROUP], F32, tag="lg")
            nc.vector.tensor_copy(lg_sb[ER:M_W, :], mid_ps[ER:M_W, :])

            logT_ps = ps_g.tile([128, TPG, E], F32, tag="gps")
            for tl in range(TPG):
                nc.tensor.matmul(
                    logT_ps[:, tl, :],
                    lg_sb[ER:M_W, tl * 128:(tl + 1) * 128],
                    id32_sb[ER:M_W, ER:M_W],
                    is_transpose=True,
                )

            gates_sb = g_pool.tile([128, TPG, E], F32, tag="gates")
            for tl in range(TPG):
                L = g_pool.tile([128, E], F32, tag="L")
                nc.vector.tensor_copy(L[:], logT_ps[:, tl, :])
                m1 = g_pool.tile([128, 1], F32, tag="m1")
                nc.vector.tensor_reduce(
                    m1[:], L[:], axis=mybir.AxisListType.X, op=mybir.AluOpType.max)
                tt = g_pool.tile([128, E], F32, tag="tt")
                nc.vector.tensor_scalar(
                    tt[:], L[:], m1[:], None, op0=mybir.AluOpType.subtract)
                z = g_pool.tile([128, E], F32, tag="z")
                nc.vector.tensor_scalar(
                    z[:], tt[:], 0.0, None, op0=mybir.AluOpType.is_equal)
                msk = g_pool.tile([128, E], F32, tag="msk")
                nc.vector.scalar_tensor_tensor(
                    msk[:], z[:], -1e30, tt[:],
                    op0=mybir.AluOpType.mult, op1=mybir.AluOpType.add)
                m2 = g_pool.tile([128, 1], F32, tag="m2")
                nc.vector.tensor_reduce(
                    m2[:], msk[:], axis=mybir.AxisListType.X, op=mybir.AluOpType.max)
                s2 = g_pool.tile([128, E], F32, tag="s2")
                nc.vector.tensor_scalar(
                    s2[:], tt[:], 2.0, m2[:],
                    op0=mybir.AluOpType.mult, op1=mybir.AluOpType.subtract)
                sg = g_pool.tile([128, E], F32, tag="sg")
                nc.scalar.activation(
                    sg[:], s2[:], mybir.ActivationFunctionType.Sigmoid)
                ge = g_pool.tile([128, E], F32, tag="ge")
                nc.vector.tensor_scalar(
                    ge[:], tt[:], m2[:], None, op0=mybir.AluOpType.is_ge)
                nc.vector.tensor_tensor(
                    gates_sb[:, tl, :], ge[:], sg[:], op=mybir.AluOpType.mult)

            # ---- next group's first transpose tile fills PE while gating runs
            if gi + 1 < N_GROUPS:
                xt_next = new_xt(PROC[gi + 1])
                emit_transpose_tile(xt_next, gi + 1, 0)

            # ---- gates -> (e,r)-expanded scale -> midTs ----
            gT_ps = ps_g.tile([E, GROUP], F32, tag="gps")
            for tl in range(TPG):
                nc.tensor.matmul(
                    gT_ps[:, tl * 128:(tl + 1) * 128],
                    gates_sb[:, tl, :],
                    id32_sb[:],
                    is_transpose=True,
                )
            gT_sb = g_pool.tile([E, GROUP], MM2_DT, tag="gT")
            nc.vector.tensor_copy(gT_sb[:], gT_ps[:])

            gexp_ps = ps_g.tile([ER, GROUP], F32, tag="gps")
            nc.tensor.matmul(gexp_ps[:], sel_sb[:], gT_sb[:])
            gexp_sb = g_pool.tile([ER, GROUP], F32, tag="gexp")
            nc.scalar.copy(gexp_sb[:], gexp_ps[:])

            midTs = mid_pool.tile([ER, GROUP], MM2_DT, tag="midTs")
            nc.vector.tensor_tensor(
                midTs[:], mid_ps[0:ER, :], gexp_sb[:], op=mybir.AluOpType.mult)

            # ---- mm2 + stores for THIS group, interleaved with the next
            # group's remaining transpose tile ----
            sq = nc.sync
            emit_mm2_tile(midTs, tok_g, 0, store_q=sq)
            if gi + 1 < N_GROUPS:
                emit_transpose_tile(xt_next, gi + 1, 1)
            emit_mm2_tile(midTs, tok_g, 1, store_q=sq)

            if gi + 1 < N_GROUPS:
                xt_cur = xt_next


_CACHED = {}


def _build_module():
    key = (MM1_DT, MM2_DT, OUT_DT)
    if key in _CACHED:
        return _CACHED[key]
    nc = bacc.Bacc("TRN2", target_bir_lowering=False, debug=False)
    x_d = nc.dram_tensor("x_in", [T_C, D], X_DT, kind="ExternalInput").ap()
    wt_d = nc.dram_tensor(
        "wt_in", [128, D_CHUNKS * M_W], MM1_DT, kind="ExternalInput").ap()
    b_d = nc.dram_tensor("ball_in", [ER, D], MM2_DT, kind="ExternalInput").ap()
    sel_d = nc.dram_tensor("sel_in", [E, ER], MM2_DT, kind="ExternalInput").ap()
    id_d = nc.dram_tensor("id_in", [128, 128], ID_DT, kind="ExternalInput").ap()
    id32_d = nc.dram_tensor(
        "id32_in", [128, 128], F32, kind="ExternalInput").ap()
    out_d = nc.dram_tensor("out", [T_C, D], OUT_DT, kind="ExternalOutput").ap()
    with tile.TileContext(nc) as tc:
        build_kernel(tc, out_d, x_d, wt_d, b_d, sel_d, id_d, id32_d)
    nc.compile()
    _CACHED[key] = nc
    return nc


def _host_weights(router_w, A, B):
    W = np.concatenate([A.reshape(ER, D), router_w], axis=0).astype(np.float32)
    # pack W.T [D, 68] into SBUF partition layout [128, 30*68]:
    # partition p, chunk c, row m  <-  W.T[c*128+p, m]
    WT = np.ascontiguousarray(
        W.T.reshape(D_CHUNKS, 128, M_W).transpose(1, 0, 2).reshape(
            128, D_CHUNKS * M_W))
    B_all = np.ascontiguousarray(
        B.transpose(0, 2, 1).reshape(ER, D)).astype(np.float32)      # [(e,r), d]
    sel = np.zeros((E, ER), np.float32)
    for e in range(E):
        sel[e, e * R:(e + 1) * R] = LORA_SCALE
    import ml_dtypes
    _np_map = {F32: np.float32, F32R: np.float32,
               BF16: ml_dtypes.bfloat16, F16: np.float16}
    ident32 = np.eye(128, dtype=np.float32)
    ident = ident32.astype(_np_map[ID_DT])
    B_all = B_all.astype(_np_map[MM2_DT])
    sel = sel.astype(_np_map[MM2_DT])
    return WT, B_all, sel, ident, ident32


def make_in_maps(x, router_w, A, B):
    flat = np.ascontiguousarray(np.asarray(x, np.float32).reshape(T_FULL, D))
    WT, B_all, sel, ident, ident32 = _host_weights(
        np.asarray(router_w, np.float32),
        np.asarray(A, np.float32),
        np.asarray(B, np.float32))
    in_maps = []
    for i in range(N_CORES):
        in_maps.append({
            "x_in": flat[i * T_C:(i + 1) * T_C],
            "wt_in": WT,
            "ball_in": B_all,
            "sel_in": sel,
            "id_in": ident,
            "id32_in": ident32,
        })
    return in_maps


def kernel(x, router_w, A, B, _results_hook=None):
    from concourse.bass_utils import run_bass_kernel_spmd

    nc = _build_module()
    in_maps = make_in_maps(x, router_w, A, B)
    res = run_bass_kernel_spmd(nc, in_maps, core_ids=list(range(N_CORES)))
    if _results_hook is not None:
        _results_hook(res)
    out = np.concatenate([res.results[i]["out"] for i in range(N_CORES)], axis=0)
    return out.astype(np.float32, copy=False).reshape(B_, S, D)


if __name__ == "__main__":
    rng = np.random.default_rng(0)
    x = rng.standard_normal((B_, S, D), dtype=np.float32)
    rw = (rng.standard_normal((E, D)) * 0.02).astype(np.float32)
    A = (rng.standard_normal((E, R, D)) * 0.02).astype(np.float32)
    Bm = (rng.standard_normal((E, D, R)) * 0.02).astype(np.float32)
    out = kernel(x, rw, A, Bm)
    print("out", out.shape, out.dtype, float(np.abs(out).max()))


# revision 113
# speedup vs baseline: 1.0559x; 1.0167x over previous
"""MoE LoRA delta kernel for Trainium2 (8 NeuronCores, data-parallel over tokens).

Computation (per token t):
    logits = x @ router_w.T                      [T, 4]
    gates  = top2-softmax(logits)                [T, 4]  (exactly 2 nonzero)
    mid    = x @ A_all.T                         [T, 64]   A_all[(e,r), d]
    delta  = (mid * expand(gates) * 4.0) @ B_all [T, D]    B_all[(e,r), d]

Kernel strategy per core (T_c = 1024 tokens, 4 groups of 256):
  - W = concat([A_all, router_w]) -> [68, D]; host packs W.T into the SBUF
    partition layout [128, 30*68] so the weight DMA moves 8KB descriptors.
  - x.T tiles produced on-chip with PE transpose-mode matmuls (fp32, exact),
    evacuated to SBUF as float32r so mm1 runs at 1 cycle/row.
  - mm1 computes [68, 256] = W @ x.T in f32r with fp32 PSUM accumulation;
    rows 64:68 are the router logits.  f32r keeps the logits accurate enough
    that the top-2 expert selection matches the fp32 reference (bf16/fp16 x
    flips 2-10 tokens on the actual dataset, each costing ~0.4 rel err, so
    x must stay 4 bytes; this pins the DMA roofline at ~92us/core).
  - Gating runs in fp32 on the accumulated logits: g_e = 1{t_e >= m2} *
    sigmoid(2*t_e - m2), t = l - max(l).
  - mm2 runs in fp16 (same 1 cycle/row on PE, halves the B_all weight DMA,
    11-bit mantissa); products accumulate in fp32 PSUM.  The output is
    stored as fp16 (halves the store DMA) and the host gather upcasts to
    fp32; measured rel err 6.8e-4 against the 2e-2 budget.
  - Schedule: the next group's transpose tiles are interleaved between this
    group's mm1/gating and mm2 so the PE never waits on the DVE/ACT gating
    chain; PSUM->SBUF evacuations alternate ACT-first/DVE; outputs are
    stored as two half-tile DMAs per 128-token tile.
  - DMA: x loads + weights on the SP HWDGE queue, output stores on the Pool
    SWDGE queue, so the (serialized, 360 GB/s) DMA engines see back-to-back
    transfers from two independent queues: zero idle between the first and
    last transfer in the cost-model timeline.
"""

import os
import sys

for _p in ("/opt/trn_rl_repo", "/root/.axon_site/_ro/trn_rl_repo"):
    if os.path.isdir(_p) and _p not in sys.path:
        sys.path.insert(0, _p)

import numpy as np
from contextlib import ExitStack

import concourse.bass as bass
import concourse.bacc as bacc
import concourse.mybir as mybir
import concourse.tile as tile

N_CORES = 8
B_, S, D = 4, 2048, 3840
T_FULL = B_ * S                 # 8192
T_C = T_FULL // N_CORES         # 1024 tokens per core
E, R = 4, 16
ER = E * R                      # 64
M_W = ER + E                    # 68 = A rows + router rows
LORA_SCALE = 16.0 / np.sqrt(16.0)   # 4.0

GROUP = 256                     # tokens per mm1 group
TPG = GROUP // 128              # token tiles per group (2)
N_GROUPS = T_C // GROUP         # 4
D_CHUNKS = D // 128             # 30
MM2_CHUNKS = [(i * 512, min(512, D - i * 512)) for i in range((D + 511) // 512)]

F32 = mybir.dt.float32
F32R = mybir.dt.float32r

BF16 = mybir.dt.bfloat16
F16 = mybir.dt.float16

# Dtype choices:
#  - mm1 f32r: 1 cyc/row on PE and keeps the router logits at ~19-bit
#    precision so the top-2 expert selection matches the fp32 reference.
#  - mm2 fp16: same PE speed, halves the B_all weight DMA, 11-bit mantissa.
#  - output fp16 on device (halves the store DMA -- the single biggest
#    traffic item after x); the host gather upcasts to fp32.  Quantization
#    adds ~5e-4 rel err against the 2e-2 budget.
_DT_MAP = {"f32": F32, "f32r": F32R, "bf16": BF16, "f16": F16}
MM1_DT = _DT_MAP[os.environ.get("MOE_MM1", "f32r")]
MM2_DT = _DT_MAP[os.environ.get("MOE_MM2", "f16")]
OUT_DT = _DT_MAP[os.environ.get("MOE_OUT", "f16")]
# x streams through the PE transposes as f32r: 1.5 cycles/row vs 2.0 for
# fp32, and numerically identical here (xt is f32r-rounded downstream
# anyway).  The identity must match (walrus rejects mixed 32/16-bit matmul
# inputs, and fp32+anything is broken in HW).
X_DT = _DT_MAP[os.environ.get("MOE_X", "f32r")]
ID_DT = _DT_MAP[os.environ.get("MOE_ID", "f32r")]


def build_kernel(tc: tile.TileContext, out_d, x_d, wt_d, b_d, sel_d, id_d,
                 id32_d):
    nc = tc.nc
    with ExitStack() as ctx:
        const_pool = ctx.enter_context(tc.tile_pool(name="const", bufs=1))
        xin_pool = ctx.enter_context(tc.tile_pool(name="xin", bufs=4))
        xt_pool = ctx.enter_context(tc.tile_pool(name="xt", bufs=2))
        mid_pool = ctx.enter_context(tc.tile_pool(name="mid", bufs=2))
        g_pool = ctx.enter_context(tc.tile_pool(name="gate", bufs=2))
        dout_pool = ctx.enter_context(tc.tile_pool(name="dout", bufs=8))
        ps_tp = ctx.enter_context(
            tc.tile_pool(name="ps_tp", bufs=2, space=bass.MemorySpace.PSUM))
        ps_mm1 = ctx.enter_context(
            tc.tile_pool(name="ps_mm1", bufs=2, space=bass.MemorySpace.PSUM))
        ps_g = ctx.enter_context(
            tc.tile_pool(name="ps_g", bufs=1, space=bass.MemorySpace.PSUM))
        ps_mm2 = ctx.enter_context(
            tc.tile_pool(name="ps_mm2", bufs=3, space=bass.MemorySpace.PSUM))

        PROC = list(range(N_GROUPS))
        PROC_TILES = [g * TPG + tl for g in PROC for tl in range(TPG)]

        # ---- prologue DMAs (SP queue): first x tile first so PE starts early
        x_sb = {}

        def load_x_seq(seq):
            t = PROC_TILES[seq]
            x_sb[t] = xin_pool.tile([128, D], X_DT, tag="xin", name=f"x_t{t}")
            for k0 in range(0, D, 512):
                w = min(512, D - k0)
                nc.sync.dma_start(
                    x_sb[t][:, k0:k0 + w],
                    x_d[t * 128:(t + 1) * 128, k0:k0 + w])

        # Issue ALL loads up front: the xin pool's WAR deps throttle them to
        # the transpose consumption pace, and a parked load always beats a
        # later-arriving store to the DMA engines, so loads finish early and
        # the back half of the bus schedule is pure stores.
        id_sb = const_pool.tile([128, 128], ID_DT, tag="ident")
        nc.sync.dma_start(id_sb[:], id_d[:])
        # fp32 identity for the small gating transposes (their data is fp32)
        id32_sb = const_pool.tile([128, 128], F32, tag="ident32")
        nc.sync.dma_start(id32_sb[:], id32_d[:])
        load_x_seq(0)
        wt_sb = const_pool.tile([128, D_CHUNKS, M_W], MM1_DT, tag="wt")
        nc.sync.dma_start(wt_sb[:], wt_d.rearrange("p (c m) -> p c m", m=M_W))
        load_x_seq(1)
        b_sb = const_pool.tile([ER, D], MM2_DT, tag="ball")
        nc.sync.dma_start(b_sb[:], b_d[:])
        sel_sb = const_pool.tile([E, ER], MM2_DT, tag="sel")
        nc.sync.dma_start(sel_sb[:], sel_d[:])
        for _seq in range(2, N_GROUPS * TPG):
            load_x_seq(_seq)

        cp_engines = [nc.scalar, nc.vector]
        xt_i = 0
        do_i = 0

        TPC = 4   # chunks per transpose-evacuation copy

        def new_xt(g):
            return xt_pool.tile(
                [128, D_CHUNKS, GROUP], MM1_DT, tag="xt", name=f"xt_g{g}")

        def emit_transpose_tile(xt_sb, gi, tl):
            """One 128-token x tile (processing position gi, tile tl) ->
            xt_sb (f32r), PE transposes + DVE/ACT evacuation."""
            nonlocal xt_i
            seq = gi * TPG + tl
            t = PROC_TILES[seq]
            for c0 in range(0, D_CHUNKS, TPC):
                ncc = min(TPC, D_CHUNKS - c0)
                tp_ps = ps_tp.tile([128, TPC, 128], X_DT, tag="tp")
                for cc in range(ncc):
                    c = c0 + cc
                    nc.tensor.transpose(
                        tp_ps[:, cc, :],
                        x_sb[t][:, c * 128:(c + 1) * 128],
                        id_sb[:],
                    )
                eng = cp_engines[xt_i % 2]; xt_i += 1
                dst = xt_sb[:, c0:c0 + ncc, tl * 128:(tl + 1) * 128]
                if eng is nc.vector:
                    eng.tensor_copy(dst, tp_ps[:, 0:ncc, :])
                else:
                    eng.copy(dst, tp_ps[:, 0:ncc, :])

        DSPLIT = 2048   # store each tile as two half-tile DMAs

        def emit_mm2_tile(midTs, tok_g_p, tl, store_q=None):
            # Early groups store on the SP queue: strict FIFO behind every
            # load, so stores can never steal bus slots mid-load-stream
            # (arrival-order arbitration otherwise stretches the last load
            # by ~25us).  Only the final group's stores use the Pool queue.
            nonlocal do_i
            store_q = store_q or nc.sync
            tok0 = tok_g_p + tl * 128
            dout_sb = dout_pool.tile([128, D], OUT_DT, tag="dout")
            for (d0, w) in MM2_CHUNKS:
                mm2_ps = ps_mm2.tile([128, 512], F32, tag="mm2")
                nc.tensor.matmul(
                    mm2_ps[:, 0:w],
                    midTs[:, tl * 128:(tl + 1) * 128],
                    b_sb[:, d0:d0 + w],
                )
                eng = cp_engines[do_i % 2]; do_i += 1
                if eng is nc.vector:
                    eng.tensor_copy(dout_sb[:, d0:d0 + w], mm2_ps[:, 0:w])
                else:
                    eng.copy(dout_sb[:, d0:d0 + w], mm2_ps[:, 0:w])
                if (d0 + w) in (1024, 2048, 3072):
                    store_q.dma_start(
                        out_d[tok0:tok0 + 128, d0 + w - 1024:d0 + w],
                        dout_sb[:, d0 + w - 1024:d0 + w])
            store_q.dma_start(
                out_d[tok0:tok0 + 128, 3072:D], dout_sb[:, 3072:D])

        xt_cur = new_xt(PROC[0])
        for tl in range(TPG):
            emit_transpose_tile(xt_cur, 0, tl)

        for gi, g in enumerate(PROC):
            tok_g = g * GROUP

            # ---- mm1: [68, GROUP] = W @ x.T (f32r, fp32 accumulation) ----
            mid_ps = ps_mm1.tile([M_W, GROUP], F32, tag="mm1")
            for c in range(D_CHUNKS):
                nc.tensor.matmul(
                    mid_ps[:],
                    wt_sb[:, c, :],
                    xt_cur[:, c, :],
                    start=(c == 0),
                    stop=(c == D_CHUNKS - 1),
                )

            # ---- gating (fp32, tokens on partitions) ----
            lg_sb = g_pool.tile([M_W, GROUP], F32, tag="lg")
            nc.vector.tensor_copy(lg_sb[ER:M_W, :], mid_ps[ER:M_W, :])

            logT_ps = ps_g.tile([128, TPG, E], F32, tag="gps")
            for tl in range(TPG):
                nc.tensor.matmul(
                    logT_ps[:, tl, :],
                    lg_sb[ER:M_W, tl * 128:(tl + 1) * 128],
                    id32_sb[ER:M_W, ER:M_W],
                    is_transpose=True,
                )

            gates_sb = g_pool.tile([128, TPG, E], F32, tag="gates")
            for tl in range(TPG):
                L = g_pool.tile([128, E], F32, tag="L")
                nc.vector.tensor_copy(L[:], logT_ps[:, tl, :])
                m1 = g_pool.tile([128, 1], F32, tag="m1")
                nc.vector.tensor_reduce(
                    m1[:], L[:], axis=mybir.AxisListType.X, op=mybir.AluOpType.max)
                tt = g_pool.tile([128, E], F32, tag="tt")
                nc.vector.tensor_scalar(
                    tt[:], L[:], m1[:], None, op0=mybir.AluOpType.subtract)
                z = g_pool.tile([128, E], F32, tag="z")
                nc.vector.tensor_scalar(
                    z[:], tt[:], 0.0, None, op0=mybir.AluOpType.is_equal)
                msk = g_pool.tile([128, E], F32, tag="msk")
                nc.vector.scalar_tensor_tensor(
                    msk[:], z[:], -1e30, tt[:],
                    op0=mybir.AluOpType.mult, op1=mybir.AluOpType.add)
                m2 = g_pool.tile([128, 1], F32, tag="m2")
                nc.vector.tensor_reduce(
                    m2[:], msk[:], axis=mybir.AxisListType.X, op=mybir.AluOpType.max)
                s2 = g_pool.tile([128, E], F32, tag="s2")
                nc.vector.tensor_scalar(
                    s2[:], tt[:], 2.0, m2[:],
                    op0=mybir.AluOpType.mult, op1=mybir.AluOpType.subtract)
                sg = g_pool.tile([128, E], F32, tag="sg")
                nc.scalar.activation(
                    sg[:], s2[:], mybir.ActivationFunctionType.Sigmoid)
                ge = g_pool.tile([128, E], F32, tag="ge")
                nc.vector.tensor_scalar(
                    ge[:], tt[:], m2[:], None, op0=mybir.AluOpType.is_ge)
                nc.vector.tensor_tensor(
                    gates_sb[:, tl, :], ge[:], sg[:], op=mybir.AluOpType.mult)

            # ---- next group's first transpose tile fills PE while gating runs
            if gi + 1 < N_GROUPS:
                xt_next = new_xt(PROC[gi + 1])
                emit_transpose_tile(xt_next, gi + 1, 0)

            # ---- gates -> (e,r)-expanded scale -> midTs ----
            gT_ps = ps_g.tile([E, GROUP], F32, tag="gps")
            for tl in range(TPG):
                nc.tensor.matmul(
                    gT_ps[:, tl * 128:(tl + 1) * 128],
                    gates_sb[:, tl, :],
                    id32_sb[:],
                    is_transpose=True,
                )
            gT_sb = g_pool.tile([E, GROUP], MM2_DT, tag="gT")
            nc.vector.tensor_copy(gT_sb[:], gT_ps[:])

            gexp_ps = ps_g.tile([ER, GROUP], F32, tag="gps")
            nc.tensor.matmul(gexp_ps[:], sel_sb[:], gT_sb[:])
            gexp_sb = g_pool.tile([ER, GROUP], F32, tag="gexp")
            nc.scalar.copy(gexp_sb[:], gexp_ps[:])

            midTs = mid_pool.tile([ER, GROUP], MM2_DT, tag="midTs")
            nc.vector.tensor_tensor(
                midTs[:], mid_ps[0:ER, :], gexp_sb[:], op=mybir.AluOpType.mult)

            # ---- mm2 + stores for THIS group, interleaved with the next
            # group's remaining transpose tile ----
            sq = nc.sync
            emit_mm2_tile(midTs, tok_g, 0, store_q=sq)
            if gi + 1 < N_GROUPS:
                emit_transpose_tile(xt_next, gi + 1, 1)
            emit_mm2_tile(midTs, tok_g, 1, store_q=sq)

            if gi + 1 < N_GROUPS:
                xt_cur = xt_next


_CACHED = {}


def _build_module():
    key = (MM1_DT, MM2_DT, OUT_DT)
    if key in _CACHED:
        return _CACHED[key]
    nc = bacc.Bacc("TRN2", target_bir_lowering=False, debug=False)
    x_d = nc.dram_tensor("x_in", [T_C, D], X_DT, kind="ExternalInput").ap()
    wt_d = nc.dram_tensor(
        "wt_in", [128, D_CHUNKS * M_W], MM1_DT, kind="ExternalInput").ap()
    b_d = nc.dram_tensor("ball_in", [ER, D], MM2_DT, kind="ExternalInput").ap()
    sel_d = nc.dram_tensor("sel_in", [E, ER], MM2_DT, kind="ExternalInput").ap()
    id_d = nc.dram_tensor("id_in", [128, 128], ID_DT, kind="ExternalInput").ap()
    id32_d = nc.dram_tensor(
        "id32_in", [128, 128], F32, kind="ExternalInput").ap()
    out_d = nc.dram_tensor("out", [T_C, D], OUT_DT, kind="ExternalOutput").ap()
    with tile.TileContext(nc) as tc:
        build_kernel(tc, out_d, x_d, wt_d, b_d, sel_d, id_d, id32_d)
    nc.compile()
    _CACHED[key] = nc
    return nc


def _host_weights(router_w, A, B):
    W = np.concatenate([A.reshape(ER, D), router_w], axis=0).astype(np.float32)
    # pack W.T [D, 68] into SBUF partition layout [128, 30*68]:
    # partition p, chunk c, row m  <-  W.T[c*128+p, m]
    WT = np.ascontiguousarray(
        W.T.reshape(D_CHUNKS, 128, M_W).transpose(1, 0, 2).reshape(
            128, D_CHUNKS * M_W))
    B_all = np.ascontiguousarray(
        B.transpose(0, 2, 1).reshape(ER, D)).astype(np.float32)      # [(e,r), d]
    sel = np.zeros((E, ER), np.float32)
    for e in range(E):
        sel[e, e * R:(e + 1) * R] = LORA_SCALE
    import ml_dtypes
    _np_map = {F32: np.float32, F32R: np.float32,
               BF16: ml_dtypes.bfloat16, F16: np.float16}
    ident32 = np.eye(128, dtype=np.float32)
    ident = ident32.astype(_np_map[ID_DT])
    B_all = B_all.astype(_np_map[MM2_DT])
    sel = sel.astype(_np_map[MM2_DT])
    return WT, B_all, sel, ident, ident32


def make_in_maps(x, router_w, A, B):
    flat = np.ascontiguousarray(np.asarray(x, np.float32).reshape(T_FULL, D))
    WT, B_all, sel, ident, ident32 = _host_weights(
        np.asarray(router_w, np.float32),
        np.asarray(A, np.float32),
        np.asarray(B, np.float32))
    in_maps = []
    for i in range(N_CORES):
        in_maps.append({
            "x_in": flat[i * T_C:(i + 1) * T_C],
            "wt_in": WT,
            "ball_in": B_all,
            "sel_in": sel,
            "id_in": ident,
            "id32_in": ident32,
        })
    return in_maps


def kernel(x, router_w, A, B, _results_hook=None):
    from concourse.bass_utils import run_bass_kernel_spmd

    nc = _build_module()
    in_maps = make_in_maps(x, router_w, A, B)
    res = run_bass_kernel_spmd(nc, in_maps, core_ids=list(range(N_CORES)))
    if _results_hook is not None:
        _results_hook(res)
    out = np.concatenate([res.results[i]["out"] for i in range(N_CORES)], axis=0)
    return out.astype(np.float32, copy=False).reshape(B_, S, D)


if __name__ == "__main__":
    rng = np.random.default_rng(0)
    x = rng.standard_normal((B_, S, D), dtype=np.float32)
    rw = (rng.standard_normal((E, D)) * 0.02).astype(np.float32)
    A = (rng.standard_normal((E, R, D)) * 0.02).astype(np.float32)
    Bm = (rng.standard_normal((E, D, R)) * 0.02).astype(np.float32)
    out = kernel(x, rw, A, Bm)
    print("out", out.shape, out.dtype, float(np.abs(out).max()))
